# revision 27
# baseline (speedup 1.0000x reference)
"""Trainium2 Bass kernel for CancerGATE (3-omics GAT autoencoder).

Sharding: nodes row-sharded across 8 NeuronCores. Dense phases (embedding,
projections, decode) run on each core's 6250-node shard in a transposed
layout (features on partitions, nodes on the free dim). The projected
features + attention-left logits are AllGathered into a per-core DRAM table;
the edge phase gathers source rows by edge (dma_gather, int16 indices, table
split in two halves), weights them by the attention coefficient and
scatter-adds into per-destination-block PSUM via host-built one-hot matmuls.
Edge softmax uses the unnormalized form (exp without max subtraction -- the
logit range for this model is [-3, 4]) so normalization is a single
reciprocal per destination node after aggregation.
"""
import sys
sys.path.insert(0, '/opt/trn_rl_repo')

import numpy as np
import ml_dtypes

import concourse.bass as bass
import concourse.bacc as bacc
import concourse.tile as tile
from concourse import mybir
from concourse.bass import IndirectOffsetOnAxis, broadcast_tensor_aps
from concourse.bass_utils import run_bass_kernel_spmd
from concourse.masks import make_identity

USE_INDIRECT = False  # HW layout of indirect gather differs from sim; using dma_gather
USE_PREP = False  # prepare_only+trigger raced on HW (NaN); keep plain gathers

F32 = mybir.dt.float32
BF16 = mybir.dt.bfloat16
F8 = mybir.dt.float8e4
I16 = mybir.dt.int16
AF = mybir.ActivationFunctionType
OP = mybir.AluOpType

P = 128
GB = 8  # tiles per dma_gather batch (1024 indices; >=2048 wedges the device)


def _dcfg(N=50000, NCORE=8, IN_DIM=512, D0=128, H=4, O0=128, O1=64, FW=512):
    c = {}
    c['N'] = N; c['NCORE'] = NCORE; c['IN_DIM'] = IN_DIM; c['D0'] = D0
    c['H'] = H; c['O0'] = O0; c['O1'] = O1
    c['CONCAT'] = 3 * D0
    c['Z0'] = H * O0
    c['Z1'] = H * O1
    c['DEC'] = c['CONCAT'] // H
    c['NLOC'] = N // NCORE
    c['NB'] = -(-c['NLOC'] // P)
    c['SPLIT'] = N // 2
    # table row widths in fp8 bytes: z (fp8) + 8 bf16 el vals (16 B),
    # padded to a 256 B multiple (dma_gather elem constraint)
    c['EXT0'] = -(-(c['Z0'] + 16) // 256) * 256
    c['EXT1'] = -(-(c['Z1'] + 16) // 256) * 256
    c['FW'] = FW
    return c


CFG = _dcfg()


def _chunks(total, w):
    return [(a, min(a + w, total)) for a in range(0, total, w)]


def _f2(ap):
    """Flatten a sliced 3D AP to 2D [P, cols]."""
    return ap.rearrange("p a b -> p (a b)")


def build_bass(cfg, segT):
    N, NCORE, NLOC, NB = cfg['N'], cfg['NCORE'], cfg['NLOC'], cfg['NB']
    IN_DIM, CONCAT, Z0, Z1, DEC = (cfg['IN_DIM'], cfg['CONCAT'], cfg['Z0'],
                                   cfg['Z1'], cfg['DEC'])
    EXT0, EXT1, H, FW = cfg['EXT0'], cfg['EXT1'], cfg['H'], cfg['FW']
    SPLIT = cfg['SPLIT']
    KI = IN_DIM // P      # k-chunks for IN_DIM contraction
    KC = CONCAT // P      # k-chunks for CONCAT
    K0 = Z0 // P          # z0 partition blocks
    K1 = Z1 // P          # z1 partition blocks
    KD = -(-DEC * H // P) # = KC
    LW = NLOC - (NB - 1) * P  # last node-block width
    nch = _chunks(NLOC, cfg['FW'])
    # per-(blk,half) segment tile counts (max over cores, host-computed)
    segT = list(segT)
    offs = [0]
    for t in segT:
        offs.append(offs[-1] + t)
    TOT = offs[-1]        # total edge tiles per core

    nc = bacc.Bacc("TRN2", target_bir_lowering=False, debug=False,
                   num_devices=NCORE)

    # ---------------- I/O ----------------
    ein = lambda nm, sh, dt: nc.dram_tensor(nm, sh, dt, kind="ExternalInput")
    featT = [ein(f"featT{i}", [IN_DIM, NLOC], BF16) for i in range(3)]
    Wemb = [ein(f"Wemb{i}", [IN_DIM, cfg['D0']], BF16) for i in range(3)]
    bemb = [ein(f"bemb{i}", [cfg['D0'], 1], F32) for i in range(3)]
    fc0 = ein("fc0", [CONCAT, Z0], BF16)
    res0 = ein("res0", [CONCAT, Z0], BF16)
    alr0 = ein("alr0", [Z0, 8], BF16)
    fc1 = ein("fc1", [Z0, Z1], BF16)
    res1 = ein("res1", [Z0, Z1], BF16)
    alr1 = ein("alr1", [Z1, 8], BF16)
    fc1T = ein("fc1T", [Z1, Z0], BF16)
    fc0T = ein("fc0T", [Z0, CONCAT], BF16)
    WdT4 = [ein(f"WdT4{i}", [CONCAT, IN_DIM], BF16) for i in range(3)]
    bd = [ein(f"bd{i}", [P, IN_DIM // P], F32) for i in range(3)]
    idx_d = ein("idx", [P, TOT * 8], I16)
    oh_d = ein("oh", [P, TOT * P], F8)
    ohT_d = ein("ohT", [P, TOT * P], BF16)
    outT = [nc.dram_tensor(f"outT{i}", [IN_DIM, NLOC], F32,
                           kind="ExternalOutput") for i in range(3)]

    with tile.TileContext(nc) as tc:
        with (
            tc.tile_pool(name="wpool", bufs=1) as wp,
            tc.tile_pool(name="dram", bufs=1, space="DRAM") as dp,
            tc.tile_pool(name="persist", bufs=1) as pp,
        ):
            # ------------- internal DRAM -------------
            aspace = "Shared" if NCORE > 4 else "Local"
            ag_in0 = dp.tile([NLOC, EXT0], F8)
            table0 = dp.tile([N, EXT0], F8, addr_space=aspace)
            ag_in1 = dp.tile([NLOC, EXT1], F8)
            table1 = dp.tile([N, EXT1], F8, addr_space=aspace)
            r0T_d = dp.tile([Z0, NLOC], BF16)
            r1T_d = dp.tile([Z1, NLOC], BF16)
            rst0_d = dp.tile([NB * P, Z0], BF16)
            rst1_d = dp.tile([NB * P, Z1], BF16)

            # ------------- persistent SBUF -------------
            ident = pp.tile([P, P], BF16)
            make_identity(nc, ident[:])
            er0_all = pp.tile([P, NB * 8], BF16)
            er1_all = pp.tile([P, NB * 8], BF16)
            nc.vector.memset(er0_all[:], 0.0)
            nc.vector.memset(er1_all[:], 0.0)

            # weights to SBUF (decode weights loaded late, in the decode
            # pool, so edge-phase pools can use the space)
            def wload(name, t, kparts, pool=None):
                w = (pool or wp).tile([P, kparts, t.shape[1]], BF16, name=name)
                nc.sync.dma_start(
                    w[:], t[:, :].rearrange("(k p) m -> p k m", p=P))
                return w
            Wemb_s = [wload(f"wemb{i}", Wemb[i], KI) for i in range(3)]
            fc0_s = wload("fc0s", fc0, KC)
            res0_s = wload("res0s", res0, KC)
            alr0_s = wload("alr0s", alr0, K0)
            fc1_s = wload("fc1s", fc1, K0)
            res1_s = wload("res1s", res1, K0)
            alr1_s = wload("alr1s", alr1, K1)
            bemb_s = []
            for i in range(3):
                b = wp.tile([P, 1], F32, name=f"bemb{i}s")
                nc.sync.dma_start(b[:], bemb[i][:, :])
                bemb_s.append(b)
            bd_s = []
            for i in range(3):
                b = wp.tile([P, IN_DIM // P], F32, name=f"bd{i}s")
                nc.sync.dma_start(b[:], bd[i][:, :])
                bd_s.append(b)

            # =========================================================
            # helper: dense matmul  outT_sb[:, mb, c0:c1] over chunks
            # =========================================================
            def dense(out_sb, sbuf_pool, lhs_sb, kparts, mblocks, rhs_sb,
                      psum_pool, act_fn, bias=None, out_f32_to=None,
                      name=""):
                """out[mb][P, chunk] = act( sum_k lhs[k].T @ rhs[k] + bias )"""
                for mb in range(mblocks):
                    for (a, b) in nch:
                        w = b - a
                        ps = psum_pool.tile([P, FW], F32, tag="dps")
                        for k in range(kparts):
                            nc.tensor.matmul(
                                ps[:, :w],
                                lhsT=lhs_sb[:, k, mb * P:(mb + 1) * P],
                                rhs=rhs_sb[:, k, a:b],
                                start=(k == 0), stop=(k == kparts - 1))
                        kw = {}
                        if bias is not None:
                            kw['bias'] = bias(mb)
                        if out_f32_to is not None:
                            o = sbuf_pool.tile([P, FW], F32, tag="dout")
                            nc.scalar.activation(o[:, :w], ps[:, :w],
                                                 act_fn, **kw)
                            nc.sync.dma_start(out_f32_to[mb * P:(mb + 1) * P,
                                                         a:b], o[:, :w])
                        else:
                            nc.scalar.activation(out_sb[:, mb, a:b],
                                                 ps[:, :w], act_fn, **kw)

            # =========================================================
            # D0: embedding -> hT  (CONCAT x NLOC, bf16, SBUF)
            # =========================================================
            with tc.tile_pool(name="d0psum", bufs=2, space="PSUM") as psp, \
                 tc.tile_pool(name="d0sb", bufs=3) as sbp:
                hT = pp.tile([P, KC, NLOC], BF16, name="hT", tag="hbuf")
                for fb in range(3):
                    for (a, b) in nch:
                        w = b - a
                        ps = psp.tile([P, FW], F32, tag="emb")
                        for k in range(KI):
                            rt = sbp.tile([P, FW], BF16, tag="feat")
                            nc.sync.dma_start(
                                rt[:, :w], featT[fb][k * P:(k + 1) * P, a:b])
                            nc.tensor.matmul(ps[:, :w],
                                             lhsT=Wemb_s[fb][:, k, :],
                                             rhs=rt[:, :w],
                                             start=(k == 0),
                                             stop=(k == KI - 1))
                        nc.scalar.activation(hT[:, fb, a:b], ps[:, :w],
                                             AF.Relu, bias=bemb_s[fb][:, :1])

            # =========================================================
            # D0b: z0T, el/er, r0T, table0 assembly, AllGather
            # =========================================================
            def proj_layer(h_sb, kparts, fc_sb, res_sb, alr_sb, zparts,
                           er_all, ag_in, table, rT_d, EXT, Z):
                with tc.tile_pool(name="p1psum", bufs=2, space="PSUM") as psp, \
                     tc.tile_pool(name="p1sb", bufs=3) as sbp, \
                     tc.tile_pool(name="p1z", bufs=1) as zp:
                    zT = zp.tile([P, zparts, NLOC], BF16, name="zT")
                    # zT = fc.T @ h
                    for zb in range(zparts):
                        for (a, b) in nch:
                            w = b - a
                            ps = psp.tile([P, FW], F32, tag="z")
                            for k in range(kparts):
                                nc.tensor.matmul(
                                    ps[:, :w],
                                    lhsT=fc_sb[:, k, zb * P:(zb + 1) * P],
                                    rhs=h_sb[:, k, a:b],
                                    start=(k == 0), stop=(k == kparts - 1))
                            nc.scalar.activation(zT[:, zb, a:b], ps[:, :w],
                                                 AF.Copy)
                    # el/er (8 rows) from zT
                    lr_hi = pp.tile([8, NLOC], BF16, name="lrhi", tag="lrhi")
                    lr_lo = pp.tile([8, NLOC], BF16, name="lrlo", tag="lrlo")
                    for (a, b) in nch:
                        w = b - a
                        ps = psp.tile([8, FW], F32, tag="lr")
                        for zb in range(zparts):
                            nc.tensor.matmul(ps[:, :w],
                                             lhsT=alr_sb[:, zb, :],
                                             rhs=zT[:, zb, a:b],
                                             start=(zb == 0),
                                             stop=(zb == zparts - 1))
                        nc.vector.tensor_copy(lr_hi[:, a:b], ps[:, :w])
                        nc.vector.tensor_tensor(lr_lo[:, a:b], ps[:, :w],
                                                lr_hi[:, a:b], op=OP.subtract)
                    # assemble node-major table rows + er_all
                    for nb in range(NB):
                        a = nb * P
                        w = min(P, NLOC - a)
                        stage = sbp.tile([P, EXT], F8, tag="stage")
                        sel = stage[:, Z:Z + 16].bitcast(BF16)  # 8 bf16 el
                        nc.vector.memset(stage[:, Z + 16:EXT], 0.0)
                        for zb in range(zparts):
                            pt = psp.tile([P, P], BF16, tag="tr")
                            nc.tensor.transpose(pt[:w, :], zT[:, zb, a:a + w],
                                                ident[:])
                            nc.vector.tensor_copy(
                                stage[:w, zb * P:(zb + 1) * P], pt[:w, :])
                        pt = psp.tile([P, 8], BF16, tag="tr")
                        nc.tensor.transpose(pt[:w, :], lr_hi[:, a:a + w],
                                            ident[:8, :8])
                        nc.vector.tensor_copy(sel[:w, 0:4], pt[:w, 0:4])
                        nc.vector.tensor_copy(er_all[:w, nb * 8:nb * 8 + 4],
                                              pt[:w, 4:8])
                        pt2 = psp.tile([P, 8], BF16, tag="tr")
                        nc.tensor.transpose(pt2[:w, :], lr_lo[:, a:a + w],
                                            ident[:8, :8])
                        nc.vector.tensor_copy(sel[:w, 4:8], pt2[:w, 0:4])
                        nc.vector.tensor_copy(
                            er_all[:w, nb * 8 + 4:nb * 8 + 8], pt2[:w, 4:8])
                        nc.sync.dma_start(ag_in[a:a + w, :], stage[:w, :])
                    nc.gpsimd.collective_compute(
                        "AllGather", OP.bypass,
                        replica_groups=[list(range(NCORE))],
                        ins=[ag_in[:]], outs=[table[:]])
                    # residual projection rT
                    for rb in range(zparts):
                        for (a, b) in nch:
                            w = b - a
                            ps = psp.tile([P, FW], F32, tag="r")
                            for k in range(kparts):
                                nc.tensor.matmul(
                                    ps[:, :w],
                                    lhsT=res_sb[:, k, rb * P:(rb + 1) * P],
                                    rhs=h_sb[:, k, a:b],
                                    start=(k == 0), stop=(k == kparts - 1))
                            ot = sbp.tile([P, FW], BF16, tag="rout")
                            nc.scalar.activation(ot[:, :w], ps[:, :w], AF.Copy)
                            nc.sync.dma_start(rT_d[rb * P:(rb + 1) * P, a:b],
                                              ot[:, :w])

            proj_layer(hT, KC, fc0_s, res0_s, alr0_s, K0, er0_all,
                       ag_in0, table0, r0T_d, EXT0, Z0)

            # =========================================================
            # E: edge phase (shared for both layers)
            # =========================================================
            def edge_phase(table, er_all, rst_d, EXT, Z, gsem):
                zparts = Z // P
                with tc.tile_pool(name="epsum", bufs=2, space="PSUM") as psp, \
                     tc.tile_pool(name="esb", bufs=2) as sbp, \
                     tc.tile_pool(name="ezg", bufs=3) as zgp, \
                     tc.tile_pool(name="esb2", bufs=2) as sbp2:
                    for blk in range(NB):
                        # psz cols [0:Z) = weighted-z accum; [Z:Z+4) = sum(u)
                        psz = psp.tile([P, Z + 4], F32, tag="psz")
                        for half in range(2):
                            seg = (blk * 2 + half)
                            T = segT[seg]
                            o = offs[seg]
                            ohs = sbp2.tile([P, T * P], F8, tag="ohs")
                            nc.sync.dma_start(
                                ohs[:], oh_d[:, o * P:(o + T) * P])
                            ohTs = sbp2.tile([P, T * P], BF16, tag="ohTs")
                            nc.sync.dma_start(
                                ohTs[:], ohT_d[:, o * P:(o + T) * P])
                            tbl = table[:SPLIT, :] if half == 0 \
                                else table[SPLIT:, :]
                            for (t0, t1) in _chunks(T, GB):
                                nt = t1 - t0
                                sz = nt * P
                                it = sbp.tile([P, GB * 8], I16, tag="idx")
                                nc.sync.dma_start(
                                    it[:, :nt * 8],
                                    idx_d[:, (o + t0) * 8:(o + t1) * 8])
                                zg = zgp.tile([P, GB, EXT], F8, tag="zg")
                                if USE_PREP:
                                    nc.gpsimd.dma_gather(
                                        zg[:, :nt, :], tbl, it[:, :nt * 8],
                                        sz, sz, EXT, prepare_only=True,
                                        sem=gsem)
                                    nc.gpsimd.trigger_dma(count=None)
                                else:
                                    nc.gpsimd.dma_gather(
                                        zg[:, :nt, :], tbl, it[:, :nt * 8],
                                        sz, sz, EXT)
                                per = psp.tile([P, GB, 8], F32, tag="per")
                                for ts in range(nt):
                                    nc.tensor.matmul(
                                        per[:, ts, :],
                                        lhsT=ohs_T_slice(ohTs, t0 + ts),
                                        rhs=er_all[:, blk * 8:blk * 8 + 8],
                                        start=True, stop=True)
                                # u computation (batched over nt tiles);
                                # el rides in the row as 8 bf16 vals at
                                # byte cols [Z:Z+16)
                                zgel = zg[:, :nt, Z:Z + 16].bitcast(BF16)
                                el = sbp.tile([P, GB, 4], F32, tag="el")
                                nc.vector.tensor_tensor(
                                    el[:, :nt, :], zgel[:, :, 0:4],
                                    zgel[:, :, 4:8], op=OP.add)
                                nc.vector.tensor_tensor(
                                    el[:, :nt, :], el[:, :nt, :],
                                    per[:, :nt, 0:4], op=OP.add)
                                nc.vector.tensor_tensor(
                                    el[:, :nt, :], el[:, :nt, :],
                                    per[:, :nt, 4:8], op=OP.add)
                                nc.vector.scalar_tensor_tensor(
                                    el[:, :nt, :], el[:, :nt, :], 0.2,
                                    el[:, :nt, :], op0=OP.mult, op1=OP.max)
                                # u: bf16 copy for the numerator weighting,
                                # fp8 copy into zg cols [Z:Z+4) so the psz
                                # matmul also accumulates sum(u) per dst.
                                uf = sbp.tile([P, GB, 4], BF16, tag="uf")
                                nc.scalar.activation(
                                    uf[:, :nt, :].rearrange("p a b -> p (a b)"),
                                    el[:, :nt, :].rearrange("p a b -> p (a b)"),
                                    AF.Exp)
                                nc.scalar.activation(
                                    zg[:, :nt, Z:Z + 4], el[:, :nt, :],
                                    AF.Exp)
                                # batched per-head weighting via 0-stride
                                # bcast, in place on the gathered tile
                                hw = Z // H
                                for h in range(H):
                                    i0, i1 = broadcast_tensor_aps(
                                        zg[:, :nt, h * hw:(h + 1) * hw],
                                        uf[:, :nt, h:h + 1])
                                    nc.vector.tensor_tensor(
                                        zg[:, :nt, h * hw:(h + 1) * hw],
                                        i0, i1, op=OP.mult)
                                for ts in range(nt):
                                    t = t0 + ts
                                    first = (half == 0 and t == 0)
                                    last = (half == 1 and t == T - 1)
                                    if Z + 4 <= 512:
                                        nc.tensor.matmul(
                                            psz[:],
                                            lhsT=ohs[:, t * P:(t + 1) * P],
                                            rhs=zg[:, ts, :Z + 4],
                                            start=first, stop=last)
                                    else:
                                        # matmul free dim caps at 512
                                        nc.tensor.matmul(
                                            psz[:, :Z],
                                            lhsT=ohs[:, t * P:(t + 1) * P],
                                            rhs=zg[:, ts, :Z],
                                            start=first, stop=last)
                                        nc.tensor.matmul(
                                            psz[:, Z:Z + 4],
                                            lhsT=ohs[:, t * P:(t + 1) * P],
                                            rhs=zg[:, ts, Z:Z + 4],
                                            start=first, stop=last)
                        # normalize + write back
                        sp = sbp.tile([P, 4], F32, tag="sp")
                        nc.vector.tensor_scalar_add(sp[:], psz[:, Z:Z + 4],
                                                    1e-9)
                        rp = sbp.tile([P, 4], F32, tag="rp")
                        nc.vector.reciprocal(rp[:], sp[:])
                        rt = sbp.tile([P, Z], BF16, tag="rstt")
                        hw = Z // H
                        for h in range(H):
                            nc.vector.tensor_scalar_mul(
                                rt[:, h * hw:(h + 1) * hw],
                                psz[:, h * hw:(h + 1) * hw],
                                rp[:, h:h + 1])
                        nc.sync.dma_start(rst_d[blk * P:(blk + 1) * P, :],
                                          rt[:])

            def ohs_T_slice(ohTs, t):
                return ohTs[:, t * P:(t + 1) * P]

            gsem0 = nc.alloc_semaphore("gsem0")
            gsem1 = nc.alloc_semaphore("gsem1")
            edge_phase(table0, er0_all, rst0_d, EXT0, Z0, gsem0)

            # =========================================================
            # T1: h1T = relu(rst0.T + r0T)   [Z0, NLOC] bf16 in SBUF
            # =========================================================
            def untranspose(rst_d, rT_d, zparts, relu, out_name):
                hT2 = pp.tile([P, zparts, NLOC], BF16, name=out_name,
                              tag="hbuf")
                with tc.tile_pool(name="tpsum", bufs=3, space="PSUM") as psp, \
                     tc.tile_pool(name="tsb", bufs=3) as sbp:
                    for nb in range(NB):
                        a = nb * P
                        w = min(P, NLOC - a)
                        ri = sbp.tile([P, zparts * P], BF16, tag="ri")
                        nc.sync.dma_start(ri[:w, :], rst_d[a:a + w, :])
                        for zb in range(zparts):
                            pt = psp.tile([P, P], BF16, tag="trp")
                            nc.tensor.transpose(
                                pt[:, :w], ri[:w, zb * P:(zb + 1) * P],
                                ident[:w, :w])
                            rr = sbp.tile([P, P], BF16, tag="rr")
                            nc.sync.dma_start(
                                rr[:, :w], rT_d[zb * P:(zb + 1) * P, a:a + w])
                            if relu:
                                nc.vector.tensor_tensor(
                                    hT2[:, zb, a:a + w], pt[:, :w],
                                    rr[:, :w], op=OP.add)
                                nc.scalar.activation(hT2[:, zb, a:a + w],
                                                     hT2[:, zb, a:a + w],
                                                     AF.Relu)
                            else:
                                nc.vector.tensor_tensor(
                                    hT2[:, zb, a:a + w], pt[:, :w],
                                    rr[:, :w], op=OP.add)
                return hT2

            h1T = untranspose(rst0_d, r0T_d, K0, True, "h1T")

            # ============== D1 + E1 ==============
            proj_layer(h1T, K0, fc1_s, res1_s, alr1_s, K1, er1_all,
                       ag_in1, table1, r1T_d, EXT1, Z1)
            edge_phase(table1, er1_all, rst1_d, EXT1, Z1, gsem1)

            # ============== decode ==============
            d0T = untranspose(rst1_d, r1T_d, K1, False, "d0T")
            with tc.tile_pool(name="decp", bufs=2, space="PSUM") as psp, \
                 tc.tile_pool(name="decs", bufs=3) as sbp, \
                 tc.tile_pool(name="dwp", bufs=1) as dwp:
                fc1T_s = wload("fc1Ts", fc1T, K1, pool=dwp)
                fc0T_s = wload("fc0Ts", fc0T, K0, pool=dwp)
                WdT4_s = [wload(f"wdt{i}", WdT4[i], KC, pool=dwp)
                          for i in range(3)]
                # relu on d0T in-place
                for zb in range(K1):
                    for (a, b) in nch:
                        nc.scalar.activation(d0T[:, zb, a:b], d0T[:, zb, a:b],
                                             AF.Relu)
                d1T = dwp.tile([P, K0, NLOC], BF16, name="d1T")
                dense(d1T, sbp, fc1T_s, K1, K0, d0T, psp, AF.Relu, name="d1")
                d2T = pp.tile([P, KC, NLOC], BF16, name="d2T", tag="hbuf")
                dense(d2T, sbp, fc0T_s, K0, KC, d1T, psp, AF.Relu, name="d2")
                for i in range(3):
                    dense(None, sbp, WdT4_s[i], KC, IN_DIM // P, d2T, psp,
                          AF.Sigmoid, bias=lambda mb, i=i: bd_s[i][:, mb:mb + 1],
                          out_f32_to=outT[i], name=f"o{i}")

    nc.compile()
    return nc


# =====================================================================
# Host side
# =====================================================================

def _host_prep(inputs, cfg):
    N, NCORE, NLOC, NB = cfg['N'], cfg['NCORE'], cfg['NLOC'], cfg['NB']
    SPLIT, H = cfg['SPLIT'], cfg['H']
    bf = ml_dtypes.bfloat16
    src = np.asarray(inputs['src']); dst = np.asarray(inputs['dst'])
    core = dst // NLOC
    dloc = dst % NLOC
    blk = dloc // P
    half = (src >= SPLIT).astype(np.int64)
    # tile count per (core, blk, half); per-segment tile counts are the max
    # over cores so the single SPMD program fits every core's stream.
    cnt = np.zeros((NCORE, NB, 2), np.int64)
    np.add.at(cnt, (core, blk, half), 1)
    segT = np.maximum(1, -(-cnt.max(axis=0) // P)).reshape(-1)  # [NB*2]
    offs = np.concatenate([[0], np.cumsum(segT)])               # tile offsets
    TOT = int(offs[-1])

    # shared (per-core identical) weights
    sh = {}
    for i in range(3):
        sh[f'Wemb{i}'] = np.ascontiguousarray(inputs[f'W_emb{i}']).astype(bf)
        sh[f'bemb{i}'] = np.asarray(inputs[f'b_emb{i}'],
                                    np.float32).reshape(-1, 1)
        wd = np.asarray(inputs[f'Wd{i}'], np.float32)
        sh[f'WdT4{i}'] = np.ascontiguousarray(
            np.concatenate([wd] * H, axis=0) * (1.0 / H)).astype(bf)
        sh[f'bd{i}'] = np.ascontiguousarray(
            np.asarray(inputs[f'bd{i}'], np.float32).reshape(-1, P).T)
    sh['fc0'] = np.asarray(inputs['fc0']).astype(bf)
    sh['res0'] = np.asarray(inputs['res0']).astype(bf)
    sh['fc1'] = np.asarray(inputs['fc1']).astype(bf)
    sh['res1'] = np.asarray(inputs['res1']).astype(bf)
    sh['fc1T'] = np.ascontiguousarray(np.asarray(inputs['fc1']).T).astype(bf)
    sh['fc0T'] = np.ascontiguousarray(np.asarray(inputs['fc0']).T).astype(bf)
    for li in (0, 1):
        al = np.asarray(inputs[f'al{li}'], np.float32)
        ar = np.asarray(inputs[f'ar{li}'], np.float32)
        Hh, D = al.shape
        blkm = np.zeros((Hh * D, 8), np.float32)
        for h in range(Hh):
            blkm[h * D:(h + 1) * D, h] = al[h]
            blkm[h * D:(h + 1) * D, 4 + h] = ar[h]
        sh[f'alr{li}'] = blkm.astype(bf)

    # per-core edge streams + one-hots
    per_core = []
    order_all = np.lexsort((dloc, half, blk, core))
    src_s = src[order_all]; dloc_s = dloc[order_all]
    blk_s = blk[order_all]; half_s = half[order_all]; core_s = core[order_all]
    core_off = np.searchsorted(core_s, np.arange(NCORE + 1))
    for c in range(NCORE):
        s0, s1 = core_off[c], core_off[c + 1]
        es, ed, eb, eh = (src_s[s0:s1], dloc_s[s0:s1], blk_s[s0:s1],
                          half_s[s0:s1])
        seg_id = eb * 2 + eh
        # position within each (blk, half) group
        grp_start = np.searchsorted(seg_id, np.arange(NB * 2 + 1))
        pos = np.arange(len(es)) - grp_start[seg_id]
        spos = offs[seg_id] * P + pos
        total = TOT * P
        idx16 = np.zeros(total, np.int16)
        localidx = np.where(eh == 0, es, es - SPLIT).astype(np.int16)
        idx16[spos] = localidx
        # wrapped idx layout per (blk,half): [16, T*8] tiled to 128 rows
        idx_arr = np.zeros((P, TOT * 8), np.int16)
        for g in range(NB * 2):
            Tg = int(segT[g])
            w = idx16[offs[g] * P:offs[g + 1] * P].reshape(Tg * 8, 16).T
            idx_arr[:, offs[g] * 8:offs[g + 1] * 8] = np.tile(w, (8, 1))
        # one-hots
        oh = np.zeros((P, total), mybir.dt.np(F8))
        ohT = np.zeros((P, total), bf)
        pp_ = spos
        t_of = pp_ // P
        e_of = pp_ % P
        dr = (ed - eb * P)
        oh[e_of, t_of * P + dr] = 1
        ohT[dr, t_of * P + e_of] = 1
        d = {'idx': idx_arr, 'oh': oh, 'ohT': ohT}
        r0, r1 = c * NLOC, (c + 1) * NLOC
        for i in range(3):
            d[f'featT{i}'] = np.ascontiguousarray(
                np.asarray(inputs[f'feat{i}'])[r0:r1].T).astype(bf)
        per_core.append(d)
    return sh, per_core, tuple(int(t) for t in segT)


_CACHE = {}


def _run(inputs, **kw):
    cfg = CFG
    sh, per_core, segT = _host_prep(inputs, cfg)
    key = ('v2', segT)
    if key not in _CACHE:
        _CACHE[key] = build_bass(cfg, segT)
    nc = _CACHE[key]
    in_maps = [{**sh, **pc} for pc in per_core]
    res = run_bass_kernel_spmd(nc, in_maps,
                               core_ids=list(range(cfg['NCORE'])), **kw)
    outs = []
    for i in range(3):
        outs.append(np.concatenate(
            [np.asarray(res.results[c][f'outT{i}'], np.float32).T
             for c in range(cfg['NCORE'])], axis=0))
    return tuple(outs), res


def kernel(**inputs):
    outs, _ = _run(inputs)
    return outs



# revision 33
# speedup vs baseline: 1.0597x; 1.0597x over previous
"""Trainium2 Bass kernel for CancerGATE (3-omics GAT autoencoder).

Sharding: nodes row-sharded across 8 NeuronCores. Dense phases (embedding,
projections, decode) run on each core's 6250-node shard in a transposed
layout (features on partitions, nodes on the free dim). The projected
features + attention-left logits are AllGathered into a per-core DRAM table;
the edge phase gathers source rows by edge (dma_gather, int16 indices, table
split in two halves), weights them by the attention coefficient and
scatter-adds into per-destination-block PSUM via host-built one-hot matmuls.
Edge softmax uses the unnormalized form (exp without max subtraction -- the
logit range for this model is [-3, 4]) so normalization is a single
reciprocal per destination node after aggregation.
"""
import sys
sys.path.insert(0, '/opt/trn_rl_repo')

import numpy as np
import ml_dtypes

import concourse.bass as bass
import concourse.bacc as bacc
import concourse.tile as tile
from concourse import mybir
from concourse.bass import IndirectOffsetOnAxis, broadcast_tensor_aps
from concourse.bass_utils import run_bass_kernel_spmd
from concourse.masks import make_identity

USE_INDIRECT = False  # HW layout of indirect gather differs from sim; using dma_gather
USE_PREP = False  # prepare_only+trigger raced on HW (NaN); plain gathers

F32 = mybir.dt.float32
BF16 = mybir.dt.bfloat16
I16 = mybir.dt.int16
AF = mybir.ActivationFunctionType
OP = mybir.AluOpType

P = 128
GB = 8  # tiles per dma_gather batch (1024 indices; >=2048 wedges the device)


def _dcfg(N=50000, NCORE=8, IN_DIM=512, D0=128, H=4, O0=128, O1=64, FW=512):
    c = {}
    c['N'] = N; c['NCORE'] = NCORE; c['IN_DIM'] = IN_DIM; c['D0'] = D0
    c['H'] = H; c['O0'] = O0; c['O1'] = O1
    c['CONCAT'] = 3 * D0
    c['Z0'] = H * O0
    c['Z1'] = H * O1
    c['DEC'] = c['CONCAT'] // H
    c['NLOC'] = N // NCORE
    c['NB'] = -(-c['NLOC'] // P)
    c['SPLIT'] = N // 2
    # table row widths (bf16 cols): z + 4 el_hi + 4 el_lo, padded to 128 cols
    c['EXT0'] = -(-(c['Z0'] + 8) // P) * P
    c['EXT1'] = -(-(c['Z1'] + 8) // P) * P
    c['FW'] = FW
    return c


CFG = _dcfg()


def _chunks(total, w):
    return [(a, min(a + w, total)) for a in range(0, total, w)]


def _f2(ap):
    """Flatten a sliced 3D AP to 2D [P, cols]."""
    return ap.rearrange("p a b -> p (a b)")


def build_bass(cfg, segT):
    N, NCORE, NLOC, NB = cfg['N'], cfg['NCORE'], cfg['NLOC'], cfg['NB']
    IN_DIM, CONCAT, Z0, Z1, DEC = (cfg['IN_DIM'], cfg['CONCAT'], cfg['Z0'],
                                   cfg['Z1'], cfg['DEC'])
    EXT0, EXT1, H, FW = cfg['EXT0'], cfg['EXT1'], cfg['H'], cfg['FW']
    SPLIT = cfg['SPLIT']
    KI = IN_DIM // P      # k-chunks for IN_DIM contraction
    KC = CONCAT // P      # k-chunks for CONCAT
    K0 = Z0 // P          # z0 partition blocks
    K1 = Z1 // P          # z1 partition blocks
    KD = -(-DEC * H // P) # = KC
    LW = NLOC - (NB - 1) * P  # last node-block width
    HALF_LOC = NLOC // 2      # AllGather chunk boundary (local rows)
    nch = _chunks(NLOC, cfg['FW'])
    # per-(blk,half) segment tile counts (max over cores, host-computed)
    segT = list(segT)
    offs = [0]
    for t in segT:
        offs.append(offs[-1] + t)
    TOT = offs[-1]        # total edge tiles per core

    nc = bacc.Bacc("TRN2", target_bir_lowering=False, debug=False,
                   num_devices=NCORE)

    # ---------------- I/O ----------------
    ein = lambda nm, sh, dt: nc.dram_tensor(nm, sh, dt, kind="ExternalInput")
    featT = [ein(f"featT{i}", [IN_DIM, NLOC], BF16) for i in range(3)]
    Wemb = [ein(f"Wemb{i}", [IN_DIM, cfg['D0']], BF16) for i in range(3)]
    bemb = [ein(f"bemb{i}", [cfg['D0'], 1], F32) for i in range(3)]
    fc0 = ein("fc0", [CONCAT, Z0], BF16)
    res0 = ein("res0", [CONCAT, Z0], BF16)
    alr0 = ein("alr0", [Z0, 8], BF16)
    fc1 = ein("fc1", [Z0, Z1], BF16)
    res1 = ein("res1", [Z0, Z1], BF16)
    alr1 = ein("alr1", [Z1, 8], BF16)
    fc1T = ein("fc1T", [Z1, Z0], BF16)
    fc0T = ein("fc0T", [Z0, CONCAT], BF16)
    WdT4 = [ein(f"WdT4{i}", [CONCAT, IN_DIM], BF16) for i in range(3)]
    bd = [ein(f"bd{i}", [P, IN_DIM // P], F32) for i in range(3)]
    idx_d = ein("idx", [P, TOT * 8], I16)
    oh_d = ein("oh", [P, TOT * P], BF16)
    ohT_d = ein("ohT", [P, TOT * P], BF16)
    outT = [nc.dram_tensor(f"outT{i}", [IN_DIM, NLOC], F32,
                           kind="ExternalOutput") for i in range(3)]

    with tile.TileContext(nc) as tc:
        with (
            tc.tile_pool(name="wpool", bufs=1) as wp,
            tc.tile_pool(name="dram", bufs=1, space="DRAM") as dp,
            tc.tile_pool(name="persist", bufs=1) as pp,
        ):
            # ------------- internal DRAM -------------
            aspace = "Shared" if NCORE > 4 else "Local"
            ag_in0 = dp.tile([NLOC, EXT0], BF16)
            table0 = [dp.tile([SPLIT, EXT0], BF16, addr_space=aspace,
                              name=f"table0_{i}") for i in range(2)]
            ag_in1 = dp.tile([NLOC, EXT1], BF16)
            table1 = [dp.tile([SPLIT, EXT1], BF16, addr_space=aspace,
                              name=f"table1_{i}") for i in range(2)]
            r0T_d = dp.tile([Z0, NLOC], BF16)
            r1T_d = dp.tile([Z1, NLOC], BF16)
            rst0_d = dp.tile([NB * P, Z0], BF16)
            rst1_d = dp.tile([NB * P, Z1], BF16)

            # ------------- persistent SBUF -------------
            ident = pp.tile([P, P], BF16)
            make_identity(nc, ident[:])
            er0_all = pp.tile([P, NB * 8], BF16)
            er1_all = pp.tile([P, NB * 8], BF16)
            nc.vector.memset(er0_all[:], 0.0)
            nc.vector.memset(er1_all[:], 0.0)

            # weights to SBUF (decode weights loaded late, in the decode
            # pool, so edge-phase pools can use the space)
            def wload(name, t, kparts, pool=None):
                w = (pool or wp).tile([P, kparts, t.shape[1]], BF16, name=name)
                nc.sync.dma_start(
                    w[:], t[:, :].rearrange("(k p) m -> p k m", p=P))
                return w
            Wemb_s = [wload(f"wemb{i}", Wemb[i], KI) for i in range(3)]
            fc0_s = wload("fc0s", fc0, KC)
            res0_s = wload("res0s", res0, KC)
            alr0_s = wload("alr0s", alr0, K0)
            fc1_s = wload("fc1s", fc1, K0)
            res1_s = wload("res1s", res1, K0)
            alr1_s = wload("alr1s", alr1, K1)
            bemb_s = []
            for i in range(3):
                b = wp.tile([P, 1], F32, name=f"bemb{i}s")
                nc.sync.dma_start(b[:], bemb[i][:, :])
                bemb_s.append(b)
            bd_s = []
            for i in range(3):
                b = wp.tile([P, IN_DIM // P], F32, name=f"bd{i}s")
                nc.sync.dma_start(b[:], bd[i][:, :])
                bd_s.append(b)

            # =========================================================
            # helper: dense matmul  outT_sb[:, mb, c0:c1] over chunks
            # =========================================================
            def dense(out_sb, sbuf_pool, lhs_sb, kparts, mblocks, rhs_sb,
                      psum_pool, act_fn, bias=None, out_f32_to=None,
                      name=""):
                """out[mb][P, chunk] = act( sum_k lhs[k].T @ rhs[k] + bias )"""
                for mb in range(mblocks):
                    for (a, b) in nch:
                        w = b - a
                        ps = psum_pool.tile([P, FW], F32, tag="dps")
                        for k in range(kparts):
                            nc.tensor.matmul(
                                ps[:, :w],
                                lhsT=lhs_sb[:, k, mb * P:(mb + 1) * P],
                                rhs=rhs_sb[:, k, a:b],
                                start=(k == 0), stop=(k == kparts - 1))
                        kw = {}
                        if bias is not None:
                            kw['bias'] = bias(mb)
                        if out_f32_to is not None:
                            o = sbuf_pool.tile([P, FW], F32, tag="dout")
                            nc.scalar.activation(o[:, :w], ps[:, :w],
                                                 act_fn, **kw)
                            nc.sync.dma_start(out_f32_to[mb * P:(mb + 1) * P,
                                                         a:b], o[:, :w])
                        else:
                            nc.scalar.activation(out_sb[:, mb, a:b],
                                                 ps[:, :w], act_fn, **kw)

            # =========================================================
            # D0: embedding -> hT  (CONCAT x NLOC, bf16, SBUF)
            # =========================================================
            with tc.tile_pool(name="d0psum", bufs=2, space="PSUM") as psp, \
                 tc.tile_pool(name="d0sb", bufs=3) as sbp:
                hT = pp.tile([P, KC, NLOC], BF16, name="hT", tag="hbuf")
                for fb in range(3):
                    for (a, b) in nch:
                        w = b - a
                        ps = psp.tile([P, FW], F32, tag="emb")
                        for k in range(KI):
                            rt = sbp.tile([P, FW], BF16, tag="feat")
                            nc.sync.dma_start(
                                rt[:, :w], featT[fb][k * P:(k + 1) * P, a:b])
                            nc.tensor.matmul(ps[:, :w],
                                             lhsT=Wemb_s[fb][:, k, :],
                                             rhs=rt[:, :w],
                                             start=(k == 0),
                                             stop=(k == KI - 1))
                        nc.scalar.activation(hT[:, fb, a:b], ps[:, :w],
                                             AF.Relu, bias=bemb_s[fb][:, :1])

            # =========================================================
            # D0b: z0T, el/er, r0T, table0 assembly, AllGather
            # =========================================================
            def proj_layer(h_sb, kparts, fc_sb, res_sb, alr_sb, zparts,
                           er_all, ag_in, table, rT_d, EXT, Z):
                with tc.tile_pool(name="p1psum", bufs=2, space="PSUM") as psp, \
                     tc.tile_pool(name="p1sb", bufs=3) as sbp, \
                     tc.tile_pool(name="p1z", bufs=1) as zp:
                    zT = zp.tile([P, zparts, NLOC], BF16, name="zT")
                    # zT = fc.T @ h
                    for zb in range(zparts):
                        for (a, b) in nch:
                            w = b - a
                            ps = psp.tile([P, FW], F32, tag="z")
                            for k in range(kparts):
                                nc.tensor.matmul(
                                    ps[:, :w],
                                    lhsT=fc_sb[:, k, zb * P:(zb + 1) * P],
                                    rhs=h_sb[:, k, a:b],
                                    start=(k == 0), stop=(k == kparts - 1))
                            nc.scalar.activation(zT[:, zb, a:b], ps[:, :w],
                                                 AF.Copy)
                    # el/er (8 rows) from zT
                    lr_hi = pp.tile([8, NLOC], BF16, name="lrhi", tag="lrhi")
                    lr_lo = pp.tile([8, NLOC], BF16, name="lrlo", tag="lrlo")
                    for (a, b) in nch:
                        w = b - a
                        ps = psp.tile([8, FW], F32, tag="lr")
                        for zb in range(zparts):
                            nc.tensor.matmul(ps[:, :w],
                                             lhsT=alr_sb[:, zb, :],
                                             rhs=zT[:, zb, a:b],
                                             start=(zb == 0),
                                             stop=(zb == zparts - 1))
                        nc.vector.tensor_copy(lr_hi[:, a:b], ps[:, :w])
                        nc.vector.tensor_tensor(lr_lo[:, a:b], ps[:, :w],
                                                lr_hi[:, a:b], op=OP.subtract)
                    # assemble node-major table rows + er_all
                    for nb in range(NB):
                        a = nb * P
                        w = min(P, NLOC - a)
                        stage = sbp.tile([P, EXT], BF16, tag="stage")
                        nc.vector.memset(stage[:, Z + 8:EXT], 0.0)
                        for zb in range(zparts):
                            pt = psp.tile([P, P], BF16, tag="tr")
                            nc.tensor.transpose(pt[:w, :], zT[:, zb, a:a + w],
                                                ident[:])
                            nc.vector.tensor_copy(
                                stage[:w, zb * P:(zb + 1) * P], pt[:w, :])
                        pt = psp.tile([P, 8], BF16, tag="tr")
                        nc.tensor.transpose(pt[:w, :], lr_hi[:, a:a + w],
                                            ident[:8, :8])
                        nc.vector.tensor_copy(stage[:w, Z:Z + 4], pt[:w, 0:4])
                        nc.vector.tensor_copy(er_all[:w, nb * 8:nb * 8 + 4],
                                              pt[:w, 4:8])
                        pt2 = psp.tile([P, 8], BF16, tag="tr")
                        nc.tensor.transpose(pt2[:w, :], lr_lo[:, a:a + w],
                                            ident[:8, :8])
                        nc.vector.tensor_copy(stage[:w, Z + 4:Z + 8],
                                              pt2[:w, 0:4])
                        nc.vector.tensor_copy(
                            er_all[:w, nb * 8 + 4:nb * 8 + 8], pt2[:w, 4:8])
                        nc.sync.dma_start(ag_in[a:a + w, :], stage[:w, :])
                        if a < HALF_LOC <= a + w:
                            # first-half table chunk: lets the edge phase
                            # start on half-0 while the rest gathers
                            nc.gpsimd.collective_compute(
                                "AllGather", OP.bypass,
                                replica_groups=[list(range(NCORE))],
                                ins=[ag_in[:HALF_LOC, :]],
                                outs=[table[0][:]])
                    nc.gpsimd.collective_compute(
                        "AllGather", OP.bypass,
                        replica_groups=[list(range(NCORE))],
                        ins=[ag_in[HALF_LOC:, :]], outs=[table[1][:]])
                    # residual projection rT
                    for rb in range(zparts):
                        for (a, b) in nch:
                            w = b - a
                            ps = psp.tile([P, FW], F32, tag="r")
                            for k in range(kparts):
                                nc.tensor.matmul(
                                    ps[:, :w],
                                    lhsT=res_sb[:, k, rb * P:(rb + 1) * P],
                                    rhs=h_sb[:, k, a:b],
                                    start=(k == 0), stop=(k == kparts - 1))
                            ot = sbp.tile([P, FW], BF16, tag="rout")
                            nc.scalar.activation(ot[:, :w], ps[:, :w], AF.Copy)
                            nc.sync.dma_start(rT_d[rb * P:(rb + 1) * P, a:b],
                                              ot[:, :w])

            proj_layer(hT, KC, fc0_s, res0_s, alr0_s, K0, er0_all,
                       ag_in0, table0, r0T_d, EXT0, Z0)

            # =========================================================
            # E: edge phase (shared for both layers)
            # =========================================================
            def edge_phase(table, er_all, rst_d, EXT, Z, gsem):
                zparts = Z // P
                with tc.tile_pool(name="epsum", bufs=2, space="PSUM") as psp, \
                     tc.tile_pool(name="esb", bufs=2) as sbp, \
                     tc.tile_pool(name="ezg", bufs=3) as zgp, \
                     tc.tile_pool(name="esb2", bufs=2) as sbp2:
                    # half-0 partial sums stash (bf16) so half-1 can run
                    # as a separate pass, overlapping the second AllGather
                    stash = pp.tile([P, NB, Z + 4], BF16, tag="stash")
                    for half in range(2):
                        for blk in range(NB):
                            psz = psp.tile([P, Z + 4], F32, tag="psz")
                            seg = (blk * 2 + half)
                            T = segT[seg]
                            o = offs[seg]
                            ohs = sbp2.tile([P, T * P], BF16, tag="ohs")
                            nc.sync.dma_start(
                                ohs[:], oh_d[:, o * P:(o + T) * P])
                            ohTs = sbp2.tile([P, T * P], BF16, tag="ohTs")
                            nc.sync.dma_start(
                                ohTs[:], ohT_d[:, o * P:(o + T) * P])
                            tbl = table[half][:]
                            for (t0, t1) in _chunks(T, GB):
                                nt = t1 - t0
                                sz = nt * P
                                it = sbp.tile([P, GB * 8], I16, tag="idx")
                                nc.sync.dma_start(
                                    it[:, :nt * 8],
                                    idx_d[:, (o + t0) * 8:(o + t1) * 8])
                                zg = zgp.tile([P, GB, EXT], BF16, tag="zg")
                                if USE_PREP:
                                    nc.gpsimd.dma_gather(
                                        zg[:, :nt, :], tbl, it[:, :nt * 8],
                                        sz, sz, EXT, prepare_only=True,
                                        sem=gsem)
                                    nc.gpsimd.trigger_dma(count=None)
                                else:
                                    nc.gpsimd.dma_gather(
                                        zg[:, :nt, :], tbl, it[:, :nt * 8],
                                        sz, sz, EXT)
                                per = psp.tile([P, GB, 8], F32, tag="per")
                                for ts in range(nt):
                                    nc.tensor.matmul(
                                        per[:, ts, :],
                                        lhsT=ohs_T_slice(ohTs, t0 + ts),
                                        rhs=er_all[:, blk * 8:blk * 8 + 8],
                                        start=True, stop=True)
                                # u computation (batched over nt tiles)
                                el = sbp.tile([P, GB, 4], F32, tag="el")
                                nc.vector.tensor_tensor(
                                    el[:, :nt, :], zg[:, :nt, Z:Z + 4],
                                    zg[:, :nt, Z + 4:Z + 8], op=OP.add)
                                nc.vector.tensor_tensor(
                                    el[:, :nt, :], el[:, :nt, :],
                                    per[:, :nt, 0:4], op=OP.add)
                                nc.vector.tensor_tensor(
                                    el[:, :nt, :], el[:, :nt, :],
                                    per[:, :nt, 4:8], op=OP.add)
                                nc.vector.scalar_tensor_tensor(
                                    el[:, :nt, :], el[:, :nt, :], 0.2,
                                    el[:, :nt, :], op0=OP.mult, op1=OP.max)
                                # u (bf16) lands in zg cols [Z:Z+4): the psz
                                # matmul over [:Z+4) then also accumulates
                                # sum(u) per dst in psz[:, Z:Z+4).
                                nc.scalar.activation(
                                    zg[:, :nt, Z:Z + 4], el[:, :nt, :],
                                    AF.Exp)
                                # batched per-head weighting via 0-stride
                                # bcast, in place on the gathered tile
                                hw = Z // H
                                for h in range(H):
                                    i0, i1 = broadcast_tensor_aps(
                                        zg[:, :nt, h * hw:(h + 1) * hw],
                                        zg[:, :nt, Z + h:Z + h + 1])
                                    nc.vector.tensor_tensor(
                                        zg[:, :nt, h * hw:(h + 1) * hw],
                                        i0, i1, op=OP.mult)
                                for ts in range(nt):
                                    t = t0 + ts
                                    first = (t == 0)
                                    last = (t == T - 1)
                                    if Z + 4 <= 512:
                                        nc.tensor.matmul(
                                            psz[:],
                                            lhsT=ohs[:, t * P:(t + 1) * P],
                                            rhs=zg[:, ts, :Z + 4],
                                            start=first, stop=last)
                                    else:
                                        # matmul free dim caps at 512
                                        nc.tensor.matmul(
                                            psz[:, :Z],
                                            lhsT=ohs[:, t * P:(t + 1) * P],
                                            rhs=zg[:, ts, :Z],
                                            start=first, stop=last)
                                        nc.tensor.matmul(
                                            psz[:, Z:Z + 4],
                                            lhsT=ohs[:, t * P:(t + 1) * P],
                                            rhs=zg[:, ts, Z:Z + 4],
                                            start=first, stop=last)
                            if half == 0:
                                # stash half-0 partials (bf16)
                                nc.vector.tensor_copy(stash[:, blk, :],
                                                      psz[:])
                                continue
                            # combine halves, normalize + write back
                            tot = sbp.tile([P, Z + 4], F32, tag="tot")
                            nc.vector.tensor_tensor(tot[:], psz[:],
                                                    stash[:, blk, :],
                                                    op=OP.add)
                            sp = sbp.tile([P, 4], F32, tag="sp")
                            nc.vector.tensor_scalar_add(sp[:],
                                                        tot[:, Z:Z + 4],
                                                        1e-9)
                            rp = sbp.tile([P, 4], F32, tag="rp")
                            nc.vector.reciprocal(rp[:], sp[:])
                            rt = sbp.tile([P, Z], BF16, tag="rstt")
                            hw = Z // H
                            for h in range(H):
                                nc.vector.tensor_scalar_mul(
                                    rt[:, h * hw:(h + 1) * hw],
                                    tot[:, h * hw:(h + 1) * hw],
                                    rp[:, h:h + 1])
                            nc.sync.dma_start(
                                rst_d[blk * P:(blk + 1) * P, :], rt[:])

            def ohs_T_slice(ohTs, t):
                return ohTs[:, t * P:(t + 1) * P]

            gsem0 = nc.alloc_semaphore("gsem0")
            gsem1 = nc.alloc_semaphore("gsem1")
            edge_phase(table0, er0_all, rst0_d, EXT0, Z0, gsem0)

            # =========================================================
            # T1: h1T = relu(rst0.T + r0T)   [Z0, NLOC] bf16 in SBUF
            # =========================================================
            def untranspose(rst_d, rT_d, zparts, relu, out_name):
                hT2 = pp.tile([P, zparts, NLOC], BF16, name=out_name,
                              tag="hbuf")
                with tc.tile_pool(name="tpsum", bufs=3, space="PSUM") as psp, \
                     tc.tile_pool(name="tsb", bufs=3) as sbp:
                    for nb in range(NB):
                        a = nb * P
                        w = min(P, NLOC - a)
                        ri = sbp.tile([P, zparts * P], BF16, tag="ri")
                        nc.sync.dma_start(ri[:w, :], rst_d[a:a + w, :])
                        for zb in range(zparts):
                            pt = psp.tile([P, P], BF16, tag="trp")
                            nc.tensor.transpose(
                                pt[:, :w], ri[:w, zb * P:(zb + 1) * P],
                                ident[:w, :w])
                            rr = sbp.tile([P, P], BF16, tag="rr")
                            nc.sync.dma_start(
                                rr[:, :w], rT_d[zb * P:(zb + 1) * P, a:a + w])
                            if relu:
                                nc.vector.tensor_tensor(
                                    hT2[:, zb, a:a + w], pt[:, :w],
                                    rr[:, :w], op=OP.add)
                                nc.scalar.activation(hT2[:, zb, a:a + w],
                                                     hT2[:, zb, a:a + w],
                                                     AF.Relu)
                            else:
                                nc.vector.tensor_tensor(
                                    hT2[:, zb, a:a + w], pt[:, :w],
                                    rr[:, :w], op=OP.add)
                return hT2

            h1T = untranspose(rst0_d, r0T_d, K0, True, "h1T")

            # ============== D1 + E1 ==============
            proj_layer(h1T, K0, fc1_s, res1_s, alr1_s, K1, er1_all,
                       ag_in1, table1, r1T_d, EXT1, Z1)
            edge_phase(table1, er1_all, rst1_d, EXT1, Z1, gsem1)

            # ============== decode ==============
            d0T = untranspose(rst1_d, r1T_d, K1, False, "d0T")
            with tc.tile_pool(name="decp", bufs=2, space="PSUM") as psp, \
                 tc.tile_pool(name="decs", bufs=3) as sbp, \
                 tc.tile_pool(name="dwp", bufs=1) as dwp:
                fc1T_s = wload("fc1Ts", fc1T, K1, pool=dwp)
                fc0T_s = wload("fc0Ts", fc0T, K0, pool=dwp)
                WdT4_s = [wload(f"wdt{i}", WdT4[i], KC, pool=dwp)
                          for i in range(3)]
                # relu on d0T in-place
                for zb in range(K1):
                    for (a, b) in nch:
                        nc.scalar.activation(d0T[:, zb, a:b], d0T[:, zb, a:b],
                                             AF.Relu)
                d1T = dwp.tile([P, K0, NLOC], BF16, name="d1T")
                dense(d1T, sbp, fc1T_s, K1, K0, d0T, psp, AF.Relu, name="d1")
                d2T = pp.tile([P, KC, NLOC], BF16, name="d2T", tag="hbuf")
                dense(d2T, sbp, fc0T_s, K0, KC, d1T, psp, AF.Relu, name="d2")
                for i in range(3):
                    dense(None, sbp, WdT4_s[i], KC, IN_DIM // P, d2T, psp,
                          AF.Sigmoid, bias=lambda mb, i=i: bd_s[i][:, mb:mb + 1],
                          out_f32_to=outT[i], name=f"o{i}")

    nc.compile()
    return nc


# =====================================================================
# Host side
# =====================================================================

def _host_prep(inputs, cfg):
    N, NCORE, NLOC, NB = cfg['N'], cfg['NCORE'], cfg['NLOC'], cfg['NB']
    SPLIT, H = cfg['SPLIT'], cfg['H']
    bf = ml_dtypes.bfloat16
    src = np.asarray(inputs['src']); dst = np.asarray(inputs['dst'])
    core = dst // NLOC
    dloc = dst % NLOC
    blk = dloc // P
    # table halves are chunked by LOCAL row (AllGather chunk = first/second
    # half of each core's rows): table row = owner*HALF_LOC + local_off
    HALF_LOC = NLOC // 2
    half = ((src % NLOC) >= HALF_LOC).astype(np.int64)
    # tile count per (core, blk, half); per-segment tile counts are the max
    # over cores so the single SPMD program fits every core's stream.
    cnt = np.zeros((NCORE, NB, 2), np.int64)
    np.add.at(cnt, (core, blk, half), 1)
    segT = np.maximum(1, -(-cnt.max(axis=0) // P)).reshape(-1)  # [NB*2]
    offs = np.concatenate([[0], np.cumsum(segT)])               # tile offsets
    TOT = int(offs[-1])

    # shared (per-core identical) weights
    sh = {}
    for i in range(3):
        sh[f'Wemb{i}'] = np.ascontiguousarray(inputs[f'W_emb{i}']).astype(bf)
        sh[f'bemb{i}'] = np.asarray(inputs[f'b_emb{i}'],
                                    np.float32).reshape(-1, 1)
        wd = np.asarray(inputs[f'Wd{i}'], np.float32)
        sh[f'WdT4{i}'] = np.ascontiguousarray(
            np.concatenate([wd] * H, axis=0) * (1.0 / H)).astype(bf)
        sh[f'bd{i}'] = np.ascontiguousarray(
            np.asarray(inputs[f'bd{i}'], np.float32).reshape(-1, P).T)
    sh['fc0'] = np.asarray(inputs['fc0']).astype(bf)
    sh['res0'] = np.asarray(inputs['res0']).astype(bf)
    sh['fc1'] = np.asarray(inputs['fc1']).astype(bf)
    sh['res1'] = np.asarray(inputs['res1']).astype(bf)
    sh['fc1T'] = np.ascontiguousarray(np.asarray(inputs['fc1']).T).astype(bf)
    sh['fc0T'] = np.ascontiguousarray(np.asarray(inputs['fc0']).T).astype(bf)
    for li in (0, 1):
        al = np.asarray(inputs[f'al{li}'], np.float32)
        ar = np.asarray(inputs[f'ar{li}'], np.float32)
        Hh, D = al.shape
        blkm = np.zeros((Hh * D, 8), np.float32)
        for h in range(Hh):
            blkm[h * D:(h + 1) * D, h] = al[h]
            blkm[h * D:(h + 1) * D, 4 + h] = ar[h]
        sh[f'alr{li}'] = blkm.astype(bf)

    # per-core edge streams + one-hots
    per_core = []
    order_all = np.lexsort((dloc, half, blk, core))
    src_s = src[order_all]; dloc_s = dloc[order_all]
    blk_s = blk[order_all]; half_s = half[order_all]; core_s = core[order_all]
    core_off = np.searchsorted(core_s, np.arange(NCORE + 1))
    for c in range(NCORE):
        s0, s1 = core_off[c], core_off[c + 1]
        es, ed, eb, eh = (src_s[s0:s1], dloc_s[s0:s1], blk_s[s0:s1],
                          half_s[s0:s1])
        seg_id = eb * 2 + eh
        # position within each (blk, half) group
        grp_start = np.searchsorted(seg_id, np.arange(NB * 2 + 1))
        pos = np.arange(len(es)) - grp_start[seg_id]
        spos = offs[seg_id] * P + pos
        total = TOT * P
        idx16 = np.zeros(total, np.int16)
        localidx = ((es // NLOC) * HALF_LOC
                    + (es % NLOC) % HALF_LOC).astype(np.int16)
        idx16[spos] = localidx
        # wrapped idx layout per (blk,half): [16, T*8] tiled to 128 rows
        idx_arr = np.zeros((P, TOT * 8), np.int16)
        for g in range(NB * 2):
            Tg = int(segT[g])
            w = idx16[offs[g] * P:offs[g + 1] * P].reshape(Tg * 8, 16).T
            idx_arr[:, offs[g] * 8:offs[g + 1] * 8] = np.tile(w, (8, 1))
        # one-hots
        oh = np.zeros((P, total), bf)
        ohT = np.zeros((P, total), bf)
        pp_ = spos
        t_of = pp_ // P
        e_of = pp_ % P
        dr = (ed - eb * P)
        oh[e_of, t_of * P + dr] = 1
        ohT[dr, t_of * P + e_of] = 1
        d = {'idx': idx_arr, 'oh': oh, 'ohT': ohT}
        r0, r1 = c * NLOC, (c + 1) * NLOC
        for i in range(3):
            d[f'featT{i}'] = np.ascontiguousarray(
                np.asarray(inputs[f'feat{i}'])[r0:r1].T).astype(bf)
        per_core.append(d)
    return sh, per_core, tuple(int(t) for t in segT)


_CACHE = {}


def _run(inputs, **kw):
    cfg = CFG
    sh, per_core, segT = _host_prep(inputs, cfg)
    key = ('v2', segT)
    if key not in _CACHE:
        _CACHE[key] = build_bass(cfg, segT)
    nc = _CACHE[key]
    in_maps = [{**sh, **pc} for pc in per_core]
    res = run_bass_kernel_spmd(nc, in_maps,
                               core_ids=list(range(cfg['NCORE'])), **kw)
    outs = []
    for i in range(3):
        outs.append(np.concatenate(
            [np.asarray(res.results[c][f'outT{i}'], np.float32).T
             for c in range(cfg['NCORE'])], axis=0))
    return tuple(outs), res


def kernel(**inputs):
    outs, _ = _run(inputs)
    return outs



# revision 39
# speedup vs baseline: 1.0771x; 1.0165x over previous
"""Trainium2 Bass kernel for CancerGATE (3-omics GAT autoencoder).

Sharding: nodes row-sharded across 8 NeuronCores. Dense phases (embedding,
projections, decode) run on each core's 6250-node shard in a transposed
layout (features on partitions, nodes on the free dim). The projected
features + attention-left logits are AllGathered into two per-core DRAM
table halves (chunked by local row so the second chunk's collective
overlaps edge processing of the first); the edge phase gathers source rows
by edge (dma_gather, int16 indices), weights them by the attention
coefficient (bf16 u written into the row's spare columns so one one-hot
matmul accumulates both the weighted-z sum and the softmax denominator)
and scatter-adds into per-destination-block PSUM via host-built one-hot
matmuls. The edge loop runs half-outer with a bf16 SBUF stash of half-0
partials. Per-(dst-block, half) edge-tile counts are the max over cores,
baked into the program (~6% less padding than a global max). Edge softmax
uses the unnormalized form (exp without max subtraction -- the logit range
for this model is [-3, 4]) so normalization is a single reciprocal per
destination node after aggregation.
"""
import sys
sys.path.insert(0, '/opt/trn_rl_repo')

import numpy as np
import ml_dtypes

import concourse.bass as bass
import concourse.bacc as bacc
import concourse.tile as tile
from concourse import mybir
from concourse.bass import IndirectOffsetOnAxis, broadcast_tensor_aps
from concourse.bass_utils import run_bass_kernel_spmd
from concourse.masks import make_identity

USE_INDIRECT = False  # HW layout of indirect gather differs from sim; using dma_gather
USE_PREP = False  # prepare_only+trigger races on HW even with cleared sems

F32 = mybir.dt.float32
BF16 = mybir.dt.bfloat16
I16 = mybir.dt.int16
AF = mybir.ActivationFunctionType
OP = mybir.AluOpType

P = 128
GB = 8  # tiles per dma_gather batch (1024 indices; >=1536 wedges the device)


def _dcfg(N=50000, NCORE=8, IN_DIM=512, D0=128, H=4, O0=128, O1=64, FW=512):
    c = {}
    c['N'] = N; c['NCORE'] = NCORE; c['IN_DIM'] = IN_DIM; c['D0'] = D0
    c['H'] = H; c['O0'] = O0; c['O1'] = O1
    c['CONCAT'] = 3 * D0
    c['Z0'] = H * O0
    c['Z1'] = H * O1
    c['DEC'] = c['CONCAT'] // H
    c['NLOC'] = N // NCORE
    c['NB'] = -(-c['NLOC'] // P)
    c['SPLIT'] = N // 2
    # table row widths (bf16 cols): z + 4 el_hi + 4 el_lo, padded to 128 cols
    c['EXT0'] = -(-(c['Z0'] + 8) // P) * P
    c['EXT1'] = -(-(c['Z1'] + 8) // P) * P
    c['FW'] = FW
    return c


CFG = _dcfg()


def _chunks(total, w):
    return [(a, min(a + w, total)) for a in range(0, total, w)]


def _f2(ap):
    """Flatten a sliced 3D AP to 2D [P, cols]."""
    return ap.rearrange("p a b -> p (a b)")


def build_bass(cfg, segT):
    N, NCORE, NLOC, NB = cfg['N'], cfg['NCORE'], cfg['NLOC'], cfg['NB']
    IN_DIM, CONCAT, Z0, Z1, DEC = (cfg['IN_DIM'], cfg['CONCAT'], cfg['Z0'],
                                   cfg['Z1'], cfg['DEC'])
    EXT0, EXT1, H, FW = cfg['EXT0'], cfg['EXT1'], cfg['H'], cfg['FW']
    SPLIT = cfg['SPLIT']
    KI = IN_DIM // P      # k-chunks for IN_DIM contraction
    KC = CONCAT // P      # k-chunks for CONCAT
    K0 = Z0 // P          # z0 partition blocks
    K1 = Z1 // P          # z1 partition blocks
    KD = -(-DEC * H // P) # = KC
    LW = NLOC - (NB - 1) * P  # last node-block width
    HALF_LOC = NLOC // 2      # AllGather chunk boundary (local rows)
    nch = _chunks(NLOC, cfg['FW'])
    # per-(blk,half) segment tile counts (max over cores, host-computed)
    segT = list(segT)
    offs = [0]
    for t in segT:
        offs.append(offs[-1] + t)
    TOT = offs[-1]        # total edge tiles per core

    nc = bacc.Bacc("TRN2", target_bir_lowering=False, debug=False,
                   num_devices=NCORE)

    # ---------------- I/O ----------------
    ein = lambda nm, sh, dt: nc.dram_tensor(nm, sh, dt, kind="ExternalInput")
    featT = [ein(f"featT{i}", [IN_DIM, NLOC], BF16) for i in range(3)]
    Wemb = [ein(f"Wemb{i}", [IN_DIM, cfg['D0']], BF16) for i in range(3)]
    bemb = [ein(f"bemb{i}", [cfg['D0'], 1], F32) for i in range(3)]
    fc0 = ein("fc0", [CONCAT, Z0], BF16)
    res0 = ein("res0", [CONCAT, Z0], BF16)
    alr0 = ein("alr0", [Z0, 8], BF16)
    fc1 = ein("fc1", [Z0, Z1], BF16)
    res1 = ein("res1", [Z0, Z1], BF16)
    alr1 = ein("alr1", [Z1, 8], BF16)
    fc1T = ein("fc1T", [Z1, Z0], BF16)
    fc0T = ein("fc0T", [Z0, CONCAT], BF16)
    WdT4 = [ein(f"WdT4{i}", [CONCAT, IN_DIM], BF16) for i in range(3)]
    bd = [ein(f"bd{i}", [P, IN_DIM // P], F32) for i in range(3)]
    idx_d = ein("idx", [P, TOT * 8], I16)
    oh_d = ein("oh", [P, TOT * P], BF16)
    ohT_d = ein("ohT", [P, TOT * P], BF16)
    outT = [nc.dram_tensor(f"outT{i}", [IN_DIM, NLOC], F32,
                           kind="ExternalOutput") for i in range(3)]

    with tile.TileContext(nc) as tc:
        with (
            tc.tile_pool(name="wpool", bufs=1) as wp,
            tc.tile_pool(name="dram", bufs=1, space="DRAM") as dp,
            tc.tile_pool(name="persist", bufs=1) as pp,
        ):
            # gather DMA-completion sems for prepare_only mode. Tile only
            # clears ITS OWN sems at kernel tail, so across NEFF executions
            # (warmup + profiled run) a user sem keeps stale counts and
            # consumer waits release early -> garbage reads. Clear at start.
            gsem0 = nc.alloc_semaphore("gsem0")
            gsem1 = nc.alloc_semaphore("gsem1")
            grng = range(min(gsem0.num, gsem1.num),
                         max(gsem0.num, gsem1.num) + 1)
            nc.gpsimd.dma_reset(grng)
            nc.gpsimd.sem_clear(grng)

            # ------------- internal DRAM -------------
            aspace = "Shared" if NCORE > 4 else "Local"
            ag_in0 = dp.tile([NLOC, EXT0], BF16)
            table0 = [dp.tile([SPLIT, EXT0], BF16, addr_space=aspace,
                              name=f"table0_{i}") for i in range(2)]
            ag_in1 = dp.tile([NLOC, EXT1], BF16)
            table1 = [dp.tile([SPLIT, EXT1], BF16, addr_space=aspace,
                              name=f"table1_{i}") for i in range(2)]
            r0T_d = dp.tile([Z0, NLOC], BF16)
            r1T_d = dp.tile([Z1, NLOC], BF16)

            # ------------- persistent SBUF -------------
            ident = pp.tile([P, P], BF16)
            make_identity(nc, ident[:])
            er0_all = pp.tile([P, NB * 8], BF16)
            er1_all = pp.tile([P, NB * 8], BF16)
            nc.vector.memset(er0_all[:], 0.0)
            nc.vector.memset(er1_all[:], 0.0)

            # weights to SBUF (decode weights loaded late, in the decode
            # pool, so edge-phase pools can use the space)
            def wload(name, t, kparts, pool=None):
                w = (pool or wp).tile([P, kparts, t.shape[1]], BF16, name=name)
                nc.sync.dma_start(
                    w[:], t[:, :].rearrange("(k p) m -> p k m", p=P))
                return w
            Wemb_s = [wload(f"wemb{i}", Wemb[i], KI) for i in range(3)]
            fc0_s = wload("fc0s", fc0, KC)
            res0_s = wload("res0s", res0, KC)
            alr0_s = wload("alr0s", alr0, K0)
            fc1_s = wload("fc1s", fc1, K0)
            res1_s = wload("res1s", res1, K0)
            alr1_s = wload("alr1s", alr1, K1)
            bemb_s = []
            for i in range(3):
                b = wp.tile([P, 1], F32, name=f"bemb{i}s")
                nc.sync.dma_start(b[:], bemb[i][:, :])
                bemb_s.append(b)
            bd_s = []
            for i in range(3):
                b = wp.tile([P, IN_DIM // P], F32, name=f"bd{i}s")
                nc.sync.dma_start(b[:], bd[i][:, :])
                bd_s.append(b)

            # =========================================================
            # helper: dense matmul  outT_sb[:, mb, c0:c1] over chunks
            # =========================================================
            def dense(out_sb, sbuf_pool, lhs_sb, kparts, mblocks, rhs_sb,
                      psum_pool, act_fn, bias=None, out_f32_to=None,
                      name=""):
                """out[mb][P, chunk] = act( sum_k lhs[k].T @ rhs[k] + bias )"""
                for mb in range(mblocks):
                    for (a, b) in nch:
                        w = b - a
                        ps = psum_pool.tile([P, FW], F32, tag="dps")
                        for k in range(kparts):
                            nc.tensor.matmul(
                                ps[:, :w],
                                lhsT=lhs_sb[:, k, mb * P:(mb + 1) * P],
                                rhs=rhs_sb[:, k, a:b],
                                start=(k == 0), stop=(k == kparts - 1))
                        kw = {}
                        if bias is not None:
                            kw['bias'] = bias(mb)
                        if out_f32_to is not None:
                            o = sbuf_pool.tile([P, FW], F32, tag="dout")
                            nc.scalar.activation(o[:, :w], ps[:, :w],
                                                 act_fn, **kw)
                            nc.sync.dma_start(out_f32_to[mb * P:(mb + 1) * P,
                                                         a:b], o[:, :w])
                        else:
                            nc.scalar.activation(out_sb[:, mb, a:b],
                                                 ps[:, :w], act_fn, **kw)

            # =========================================================
            # D0: embedding -> hT  (CONCAT x NLOC, bf16, SBUF)
            # =========================================================
            with tc.tile_pool(name="d0psum", bufs=2, space="PSUM") as psp, \
                 tc.tile_pool(name="d0sb", bufs=3) as sbp:
                hT = pp.tile([P, KC, NLOC], BF16, name="hT", tag="hbuf")
                for fb in range(3):
                    for (a, b) in nch:
                        w = b - a
                        ps = psp.tile([P, FW], F32, tag="emb")
                        for k in range(KI):
                            rt = sbp.tile([P, FW], BF16, tag="feat")
                            nc.sync.dma_start(
                                rt[:, :w], featT[fb][k * P:(k + 1) * P, a:b])
                            nc.tensor.matmul(ps[:, :w],
                                             lhsT=Wemb_s[fb][:, k, :],
                                             rhs=rt[:, :w],
                                             start=(k == 0),
                                             stop=(k == KI - 1))
                        nc.scalar.activation(hT[:, fb, a:b], ps[:, :w],
                                             AF.Relu, bias=bemb_s[fb][:, :1])

            # =========================================================
            # D0b: z0T, el/er, r0T, table0 assembly, AllGather
            # =========================================================
            def proj_layer(h_sb, kparts, fc_sb, res_sb, alr_sb, zparts,
                           er_all, ag_in, table, rT_d, EXT, Z):
                with tc.tile_pool(name="p1psum", bufs=2, space="PSUM") as psp, \
                     tc.tile_pool(name="p1sb", bufs=3) as sbp, \
                     tc.tile_pool(name="p1z", bufs=1) as zp:
                    zT = zp.tile([P, zparts, NLOC], BF16, name="zT")
                    # zT = fc.T @ h
                    for zb in range(zparts):
                        for (a, b) in nch:
                            w = b - a
                            ps = psp.tile([P, FW], F32, tag="z")
                            for k in range(kparts):
                                nc.tensor.matmul(
                                    ps[:, :w],
                                    lhsT=fc_sb[:, k, zb * P:(zb + 1) * P],
                                    rhs=h_sb[:, k, a:b],
                                    start=(k == 0), stop=(k == kparts - 1))
                            nc.scalar.activation(zT[:, zb, a:b], ps[:, :w],
                                                 AF.Copy)
                    # el/er (8 rows) from zT
                    lr_hi = pp.tile([8, NLOC], BF16, name="lrhi", tag="lrhi")
                    lr_lo = pp.tile([8, NLOC], BF16, name="lrlo", tag="lrlo")
                    for (a, b) in nch:
                        w = b - a
                        ps = psp.tile([8, FW], F32, tag="lr")
                        for zb in range(zparts):
                            nc.tensor.matmul(ps[:, :w],
                                             lhsT=alr_sb[:, zb, :],
                                             rhs=zT[:, zb, a:b],
                                             start=(zb == 0),
                                             stop=(zb == zparts - 1))
                        nc.vector.tensor_copy(lr_hi[:, a:b], ps[:, :w])
                        nc.vector.tensor_tensor(lr_lo[:, a:b], ps[:, :w],
                                                lr_hi[:, a:b], op=OP.subtract)
                    # assemble node-major table rows + er_all
                    for nb in range(NB):
                        a = nb * P
                        w = min(P, NLOC - a)
                        stage = sbp.tile([P, EXT], BF16, tag="stage")
                        nc.vector.memset(stage[:, Z + 8:EXT], 0.0)
                        for zb in range(zparts):
                            pt = psp.tile([P, P], BF16, tag="tr")
                            nc.tensor.transpose(pt[:w, :], zT[:, zb, a:a + w],
                                                ident[:])
                            nc.vector.tensor_copy(
                                stage[:w, zb * P:(zb + 1) * P], pt[:w, :])
                        pt = psp.tile([P, 8], BF16, tag="tr")
                        nc.tensor.transpose(pt[:w, :], lr_hi[:, a:a + w],
                                            ident[:8, :8])
                        nc.vector.tensor_copy(stage[:w, Z:Z + 4], pt[:w, 0:4])
                        nc.vector.tensor_copy(er_all[:w, nb * 8:nb * 8 + 4],
                                              pt[:w, 4:8])
                        pt2 = psp.tile([P, 8], BF16, tag="tr")
                        nc.tensor.transpose(pt2[:w, :], lr_lo[:, a:a + w],
                                            ident[:8, :8])
                        nc.vector.tensor_copy(stage[:w, Z + 4:Z + 8],
                                              pt2[:w, 0:4])
                        nc.vector.tensor_copy(
                            er_all[:w, nb * 8 + 4:nb * 8 + 8], pt2[:w, 4:8])
                        nc.sync.dma_start(ag_in[a:a + w, :], stage[:w, :])
                        if a < HALF_LOC <= a + w:
                            # first-half table chunk: lets the edge phase
                            # start on half-0 while the rest gathers
                            nc.gpsimd.collective_compute(
                                "AllGather", OP.bypass,
                                replica_groups=[list(range(NCORE))],
                                ins=[ag_in[:HALF_LOC, :]],
                                outs=[table[0][:]])
                    nc.gpsimd.collective_compute(
                        "AllGather", OP.bypass,
                        replica_groups=[list(range(NCORE))],
                        ins=[ag_in[HALF_LOC:, :]], outs=[table[1][:]])
                    # residual projection rT
                    for rb in range(zparts):
                        for (a, b) in nch:
                            w = b - a
                            ps = psp.tile([P, FW], F32, tag="r")
                            for k in range(kparts):
                                nc.tensor.matmul(
                                    ps[:, :w],
                                    lhsT=res_sb[:, k, rb * P:(rb + 1) * P],
                                    rhs=h_sb[:, k, a:b],
                                    start=(k == 0), stop=(k == kparts - 1))
                            ot = sbp.tile([P, FW], BF16, tag="rout")
                            nc.scalar.activation(ot[:, :w], ps[:, :w], AF.Copy)
                            nc.sync.dma_start(rT_d[rb * P:(rb + 1) * P, a:b],
                                              ot[:, :w])

            proj_layer(hT, KC, fc0_s, res0_s, alr0_s, K0, er0_all,
                       ag_in0, table0, r0T_d, EXT0, Z0)

            # =========================================================
            # E: edge phase (shared for both layers)
            # =========================================================
            def edge_phase(table, er_all, rT_d, h_out, relu, EXT, Z, gsem):
                zparts = Z // P
                with tc.tile_pool(name="epsum", bufs=2, space="PSUM") as psp, \
                     tc.tile_pool(name="esb", bufs=2) as sbp, \
                     tc.tile_pool(name="ezg", bufs=3) as zgp, \
                     tc.tile_pool(name="esb2", bufs=2) as sbp2:
                    # half-0 partial sums stash (bf16) so half-1 can run
                    # as a separate pass, overlapping the second AllGather
                    stash = pp.tile([P, NB, Z + 4], BF16, tag="stash")
                    for half in range(2):
                        for blk in range(NB):
                            psz = psp.tile([P, Z + 4], F32, tag="psz")
                            seg = (blk * 2 + half)
                            T = segT[seg]
                            o = offs[seg]
                            ohs = sbp2.tile([P, T * P], BF16, tag="ohs")
                            nc.sync.dma_start(
                                ohs[:], oh_d[:, o * P:(o + T) * P])
                            ohTs = sbp2.tile([P, T * P], BF16, tag="ohTs")
                            nc.sync.dma_start(
                                ohTs[:], ohT_d[:, o * P:(o + T) * P])
                            tbl = table[half][:]
                            for (t0, t1) in _chunks(T, GB):
                                nt = t1 - t0
                                sz = nt * P
                                it = sbp.tile([P, GB * 8], I16, tag="idx")
                                nc.sync.dma_start(
                                    it[:, :nt * 8],
                                    idx_d[:, (o + t0) * 8:(o + t1) * 8])
                                zg = zgp.tile([P, GB, EXT], BF16, tag="zg")
                                if USE_PREP:
                                    nc.gpsimd.dma_gather(
                                        zg[:, :nt, :], tbl, it[:, :nt * 8],
                                        sz, sz, EXT, prepare_only=True,
                                        sem=gsem)
                                    nc.gpsimd.trigger_dma(count=None)
                                else:
                                    nc.gpsimd.dma_gather(
                                        zg[:, :nt, :], tbl, it[:, :nt * 8],
                                        sz, sz, EXT)
                                per = psp.tile([P, GB, 8], F32, tag="per")
                                for ts in range(nt):
                                    nc.tensor.matmul(
                                        per[:, ts, :],
                                        lhsT=ohs_T_slice(ohTs, t0 + ts),
                                        rhs=er_all[:, blk * 8:blk * 8 + 8],
                                        start=True, stop=True)
                                # u computation (batched over nt tiles)
                                el = sbp.tile([P, GB, 4], F32, tag="el")
                                nc.vector.tensor_tensor(
                                    el[:, :nt, :], zg[:, :nt, Z:Z + 4],
                                    zg[:, :nt, Z + 4:Z + 8], op=OP.add)
                                nc.vector.tensor_tensor(
                                    el[:, :nt, :], el[:, :nt, :],
                                    per[:, :nt, 0:4], op=OP.add)
                                nc.vector.tensor_tensor(
                                    el[:, :nt, :], el[:, :nt, :],
                                    per[:, :nt, 4:8], op=OP.add)
                                nc.vector.scalar_tensor_tensor(
                                    el[:, :nt, :], el[:, :nt, :], 0.2,
                                    el[:, :nt, :], op0=OP.mult, op1=OP.max)
                                # u (bf16) lands in zg cols [Z:Z+4): the psz
                                # matmul over [:Z+4) then also accumulates
                                # sum(u) per dst in psz[:, Z:Z+4).
                                nc.scalar.activation(
                                    zg[:, :nt, Z:Z + 4], el[:, :nt, :],
                                    AF.Exp)
                                # batched per-head weighting via 0-stride
                                # bcast, in place on the gathered tile
                                hw = Z // H
                                for h in range(H):
                                    i0, i1 = broadcast_tensor_aps(
                                        zg[:, :nt, h * hw:(h + 1) * hw],
                                        zg[:, :nt, Z + h:Z + h + 1])
                                    nc.vector.tensor_tensor(
                                        zg[:, :nt, h * hw:(h + 1) * hw],
                                        i0, i1, op=OP.mult)
                                for ts in range(nt):
                                    t = t0 + ts
                                    first = (t == 0)
                                    last = (t == T - 1)
                                    if Z + 4 <= 512:
                                        nc.tensor.matmul(
                                            psz[:],
                                            lhsT=ohs[:, t * P:(t + 1) * P],
                                            rhs=zg[:, ts, :Z + 4],
                                            start=first, stop=last)
                                    else:
                                        # matmul free dim caps at 512
                                        nc.tensor.matmul(
                                            psz[:, :Z],
                                            lhsT=ohs[:, t * P:(t + 1) * P],
                                            rhs=zg[:, ts, :Z],
                                            start=first, stop=last)
                                        nc.tensor.matmul(
                                            psz[:, Z:Z + 4],
                                            lhsT=ohs[:, t * P:(t + 1) * P],
                                            rhs=zg[:, ts, Z:Z + 4],
                                            start=first, stop=last)
                            if half == 0:
                                # stash half-0 partials (bf16)
                                nc.vector.tensor_copy(stash[:, blk, :],
                                                      psz[:])
                                continue
                            # combine halves, normalize + write back
                            tot = sbp.tile([P, Z + 4], F32, tag="tot")
                            nc.vector.tensor_tensor(tot[:], psz[:],
                                                    stash[:, blk, :],
                                                    op=OP.add)
                            sp = sbp.tile([P, 4], F32, tag="sp")
                            nc.vector.tensor_scalar_add(sp[:],
                                                        tot[:, Z:Z + 4],
                                                        1e-9)
                            rp = sbp.tile([P, 4], F32, tag="rp")
                            nc.vector.reciprocal(rp[:], sp[:])
                            rt = sbp.tile([P, Z], BF16, tag="rstt")
                            hw = Z // H
                            for h in range(H):
                                nc.vector.tensor_scalar_mul(
                                    rt[:, h * hw:(h + 1) * hw],
                                    tot[:, h * hw:(h + 1) * hw],
                                    rp[:, h:h + 1])
                            # fused untranspose: write this block straight
                            # into the next phase's [Z, NLOC] activation so
                            # downstream column-chunks can start while later
                            # blocks are still aggregating
                            a = blk * P
                            w = min(P, NLOC - a)
                            for zb in range(zparts):
                                pt = psp.tile([P, P], BF16, tag="trp")
                                nc.tensor.transpose(
                                    pt[:, :w], rt[:w, zb * P:(zb + 1) * P],
                                    ident[:w, :w])
                                rr = sbp.tile([P, P], BF16, tag="rr")
                                nc.sync.dma_start(
                                    rr[:, :w],
                                    rT_d[zb * P:(zb + 1) * P, a:a + w])
                                nc.vector.tensor_tensor(
                                    h_out[:, zb, a:a + w], pt[:, :w],
                                    rr[:, :w], op=OP.add)
                                if relu:
                                    nc.scalar.activation(
                                        h_out[:, zb, a:a + w],
                                        h_out[:, zb, a:a + w], AF.Relu)

            def ohs_T_slice(ohTs, t):
                return ohTs[:, t * P:(t + 1) * P]

            h1T = pp.tile([P, K0, NLOC], BF16, name="h1T", tag="hbuf")
            edge_phase(table0, er0_all, r0T_d, h1T, True, EXT0, Z0, gsem0)

            # ============== D1 + E1 ==============
            proj_layer(h1T, K0, fc1_s, res1_s, alr1_s, K1, er1_all,
                       ag_in1, table1, r1T_d, EXT1, Z1)
            d0T = pp.tile([P, K1, NLOC], BF16, name="d0T", tag="hbuf")
            edge_phase(table1, er1_all, r1T_d, d0T, False, EXT1, Z1, gsem1)

            # ============== decode ==============
            with tc.tile_pool(name="decp", bufs=2, space="PSUM") as psp, \
                 tc.tile_pool(name="decs", bufs=3) as sbp, \
                 tc.tile_pool(name="dwp", bufs=1) as dwp:
                fc1T_s = wload("fc1Ts", fc1T, K1, pool=dwp)
                fc0T_s = wload("fc0Ts", fc0T, K0, pool=dwp)
                WdT4_s = [wload(f"wdt{i}", WdT4[i], KC, pool=dwp)
                          for i in range(3)]
                # relu on d0T in-place
                for zb in range(K1):
                    for (a, b) in nch:
                        nc.scalar.activation(d0T[:, zb, a:b], d0T[:, zb, a:b],
                                             AF.Relu)
                d1T = dwp.tile([P, K0, NLOC], BF16, name="d1T")
                dense(d1T, sbp, fc1T_s, K1, K0, d0T, psp, AF.Relu, name="d1")
                d2T = pp.tile([P, KC, NLOC], BF16, name="d2T", tag="hbuf")
                dense(d2T, sbp, fc0T_s, K0, KC, d1T, psp, AF.Relu, name="d2")
                for i in range(3):
                    dense(None, sbp, WdT4_s[i], KC, IN_DIM // P, d2T, psp,
                          AF.Sigmoid, bias=lambda mb, i=i: bd_s[i][:, mb:mb + 1],
                          out_f32_to=outT[i], name=f"o{i}")

    nc.compile()
    return nc


# =====================================================================
# Host side
# =====================================================================

def _host_prep(inputs, cfg):
    N, NCORE, NLOC, NB = cfg['N'], cfg['NCORE'], cfg['NLOC'], cfg['NB']
    SPLIT, H = cfg['SPLIT'], cfg['H']
    bf = ml_dtypes.bfloat16
    src = np.asarray(inputs['src']); dst = np.asarray(inputs['dst'])
    core = dst // NLOC
    dloc = dst % NLOC
    blk = dloc // P
    # table halves are chunked by LOCAL row (AllGather chunk = first/second
    # half of each core's rows): table row = owner*HALF_LOC + local_off
    HALF_LOC = NLOC // 2
    half = ((src % NLOC) >= HALF_LOC).astype(np.int64)
    # tile count per (core, blk, half); per-segment tile counts are the max
    # over cores so the single SPMD program fits every core's stream.
    cnt = np.zeros((NCORE, NB, 2), np.int64)
    np.add.at(cnt, (core, blk, half), 1)
    segT = np.maximum(1, -(-cnt.max(axis=0) // P)).reshape(-1)  # [NB*2]
    offs = np.concatenate([[0], np.cumsum(segT)])               # tile offsets
    TOT = int(offs[-1])

    # shared (per-core identical) weights
    sh = {}
    for i in range(3):
        sh[f'Wemb{i}'] = np.ascontiguousarray(inputs[f'W_emb{i}']).astype(bf)
        sh[f'bemb{i}'] = np.asarray(inputs[f'b_emb{i}'],
                                    np.float32).reshape(-1, 1)
        wd = np.asarray(inputs[f'Wd{i}'], np.float32)
        sh[f'WdT4{i}'] = np.ascontiguousarray(
            np.concatenate([wd] * H, axis=0) * (1.0 / H)).astype(bf)
        sh[f'bd{i}'] = np.ascontiguousarray(
            np.asarray(inputs[f'bd{i}'], np.float32).reshape(-1, P).T)
    sh['fc0'] = np.asarray(inputs['fc0']).astype(bf)
    sh['res0'] = np.asarray(inputs['res0']).astype(bf)
    sh['fc1'] = np.asarray(inputs['fc1']).astype(bf)
    sh['res1'] = np.asarray(inputs['res1']).astype(bf)
    sh['fc1T'] = np.ascontiguousarray(np.asarray(inputs['fc1']).T).astype(bf)
    sh['fc0T'] = np.ascontiguousarray(np.asarray(inputs['fc0']).T).astype(bf)
    for li in (0, 1):
        al = np.asarray(inputs[f'al{li}'], np.float32)
        ar = np.asarray(inputs[f'ar{li}'], np.float32)
        Hh, D = al.shape
        blkm = np.zeros((Hh * D, 8), np.float32)
        for h in range(Hh):
            blkm[h * D:(h + 1) * D, h] = al[h]
            blkm[h * D:(h + 1) * D, 4 + h] = ar[h]
        sh[f'alr{li}'] = blkm.astype(bf)

    # per-core edge streams + one-hots
    per_core = []
    order_all = np.lexsort((dloc, half, blk, core))
    src_s = src[order_all]; dloc_s = dloc[order_all]
    blk_s = blk[order_all]; half_s = half[order_all]; core_s = core[order_all]
    core_off = np.searchsorted(core_s, np.arange(NCORE + 1))
    for c in range(NCORE):
        s0, s1 = core_off[c], core_off[c + 1]
        es, ed, eb, eh = (src_s[s0:s1], dloc_s[s0:s1], blk_s[s0:s1],
                          half_s[s0:s1])
        seg_id = eb * 2 + eh
        # position within each (blk, half) group
        grp_start = np.searchsorted(seg_id, np.arange(NB * 2 + 1))
        pos = np.arange(len(es)) - grp_start[seg_id]
        spos = offs[seg_id] * P + pos
        total = TOT * P
        idx16 = np.zeros(total, np.int16)
        localidx = ((es // NLOC) * HALF_LOC
                    + (es % NLOC) % HALF_LOC).astype(np.int16)
        idx16[spos] = localidx
        # wrapped idx layout per (blk,half): [16, T*8] tiled to 128 rows
        idx_arr = np.zeros((P, TOT * 8), np.int16)
        for g in range(NB * 2):
            Tg = int(segT[g])
            w = idx16[offs[g] * P:offs[g + 1] * P].reshape(Tg * 8, 16).T
            idx_arr[:, offs[g] * 8:offs[g + 1] * 8] = np.tile(w, (8, 1))
        # one-hots
        oh = np.zeros((P, total), bf)
        ohT = np.zeros((P, total), bf)
        pp_ = spos
        t_of = pp_ // P
        e_of = pp_ % P
        dr = (ed - eb * P)
        oh[e_of, t_of * P + dr] = 1
        ohT[dr, t_of * P + e_of] = 1
        d = {'idx': idx_arr, 'oh': oh, 'ohT': ohT}
        r0, r1 = c * NLOC, (c + 1) * NLOC
        for i in range(3):
            d[f'featT{i}'] = np.ascontiguousarray(
                np.asarray(inputs[f'feat{i}'])[r0:r1].T).astype(bf)
        per_core.append(d)
    return sh, per_core, tuple(int(t) for t in segT)


_CACHE = {}


def _run(inputs, **kw):
    cfg = CFG
    sh, per_core, segT = _host_prep(inputs, cfg)
    key = ('v2', segT)
    if key not in _CACHE:
        _CACHE[key] = build_bass(cfg, segT)
    nc = _CACHE[key]
    in_maps = [{**sh, **pc} for pc in per_core]
    res = run_bass_kernel_spmd(nc, in_maps,
                               core_ids=list(range(cfg['NCORE'])), **kw)
    outs = []
    for i in range(3):
        outs.append(np.concatenate(
            [np.asarray(res.results[c][f'outT{i}'], np.float32).T
             for c in range(cfg['NCORE'])], axis=0))
    return tuple(outs), res


def kernel(**inputs):
    outs, _ = _run(inputs)
    return outs



# revision 41
# speedup vs baseline: 1.0809x; 1.0035x over previous
"""Trainium2 Bass kernel for CancerGATE (3-omics GAT autoencoder).

Sharding: nodes row-sharded across 8 NeuronCores. Dense phases (embedding,
projections, decode) run on each core's 6250-node shard in a transposed
layout (features on partitions, nodes on the free dim). The projected
features + attention-left logits are AllGathered into two per-core DRAM
table halves (chunked by local row so the second chunk's collective
overlaps edge processing of the first); the edge phase gathers source rows
by edge (dma_gather, int16 indices), weights them by the attention
coefficient (bf16 u written into the row's spare columns so one one-hot
matmul accumulates both the weighted-z sum and the softmax denominator)
and scatter-adds into per-destination-block PSUM via host-built one-hot
matmuls. The edge loop runs half-outer with a bf16 SBUF stash of half-0
partials. Per-(dst-block, half) edge-tile counts are the max over cores,
baked into the program (~6% less padding than a global max). Edge softmax
uses the unnormalized form (exp without max subtraction -- the logit range
for this model is [-3, 4]) so normalization is a single reciprocal per
destination node after aggregation.
"""
import sys
sys.path.insert(0, '/opt/trn_rl_repo')

import numpy as np
import ml_dtypes

import concourse.bass as bass
import concourse.bacc as bacc
import concourse.tile as tile
from concourse import mybir
from concourse.bass import IndirectOffsetOnAxis, broadcast_tensor_aps
from concourse.bass_utils import run_bass_kernel_spmd
from concourse.masks import make_identity

USE_INDIRECT = False  # HW layout of indirect gather differs from sim; using dma_gather
USE_PREP = False  # prepare_only+trigger races on HW even with cleared sems

F32 = mybir.dt.float32
BF16 = mybir.dt.bfloat16
I16 = mybir.dt.int16
AF = mybir.ActivationFunctionType
OP = mybir.AluOpType

P = 128
GB = 8  # tiles per dma_gather batch (1024 indices; >=1536 wedges the device)


def _dcfg(N=50000, NCORE=8, IN_DIM=512, D0=128, H=4, O0=128, O1=64, FW=512):
    c = {}
    c['N'] = N; c['NCORE'] = NCORE; c['IN_DIM'] = IN_DIM; c['D0'] = D0
    c['H'] = H; c['O0'] = O0; c['O1'] = O1
    c['CONCAT'] = 3 * D0
    c['Z0'] = H * O0
    c['Z1'] = H * O1
    c['DEC'] = c['CONCAT'] // H
    c['NLOC'] = N // NCORE
    c['NB'] = -(-c['NLOC'] // P)
    c['SPLIT'] = N // 2
    # table row widths (bf16 cols): z + 4 el_hi + 4 el_lo, padded to 128 cols
    c['EXT0'] = -(-(c['Z0'] + 8) // P) * P
    c['EXT1'] = -(-(c['Z1'] + 8) // P) * P
    c['FW'] = FW
    return c


CFG = _dcfg()


def _chunks(total, w):
    return [(a, min(a + w, total)) for a in range(0, total, w)]


def _f2(ap):
    """Flatten a sliced 3D AP to 2D [P, cols]."""
    return ap.rearrange("p a b -> p (a b)")


def build_bass(cfg, segT):
    N, NCORE, NLOC, NB = cfg['N'], cfg['NCORE'], cfg['NLOC'], cfg['NB']
    IN_DIM, CONCAT, Z0, Z1, DEC = (cfg['IN_DIM'], cfg['CONCAT'], cfg['Z0'],
                                   cfg['Z1'], cfg['DEC'])
    EXT0, EXT1, H, FW = cfg['EXT0'], cfg['EXT1'], cfg['H'], cfg['FW']
    SPLIT = cfg['SPLIT']
    KI = IN_DIM // P      # k-chunks for IN_DIM contraction
    KC = CONCAT // P      # k-chunks for CONCAT
    K0 = Z0 // P          # z0 partition blocks
    K1 = Z1 // P          # z1 partition blocks
    KD = -(-DEC * H // P) # = KC
    LW = NLOC - (NB - 1) * P  # last node-block width
    HALF_LOC = NLOC // 2      # AllGather chunk boundary (local rows)
    nch = _chunks(NLOC, cfg['FW'])
    # per-(blk,half) segment tile counts (max over cores, host-computed)
    segT = list(segT)
    offs = [0]
    for t in segT:
        offs.append(offs[-1] + t)
    TOT = offs[-1]        # total edge tiles per core

    nc = bacc.Bacc("TRN2", target_bir_lowering=False, debug=False,
                   num_devices=NCORE)

    # ---------------- I/O ----------------
    ein = lambda nm, sh, dt: nc.dram_tensor(nm, sh, dt, kind="ExternalInput")
    featT = [ein(f"featT{i}", [IN_DIM, NLOC], BF16) for i in range(3)]
    Wemb = [ein(f"Wemb{i}", [IN_DIM, cfg['D0']], BF16) for i in range(3)]
    bemb = [ein(f"bemb{i}", [cfg['D0'], 1], F32) for i in range(3)]
    fc0 = ein("fc0", [CONCAT, Z0], BF16)
    res0 = ein("res0", [CONCAT, Z0], BF16)
    alr0 = ein("alr0", [Z0, 8], BF16)
    fc1 = ein("fc1", [Z0, Z1], BF16)
    res1 = ein("res1", [Z0, Z1], BF16)
    alr1 = ein("alr1", [Z1, 8], BF16)
    fc1T = ein("fc1T", [Z1, Z0], BF16)
    fc0T = ein("fc0T", [Z0, CONCAT], BF16)
    WdT4 = [ein(f"WdT4{i}", [CONCAT, IN_DIM], BF16) for i in range(3)]
    bd = [ein(f"bd{i}", [P, IN_DIM // P], F32) for i in range(3)]
    idx_d = ein("idx", [P, TOT * 8], I16)
    oh_d = ein("oh", [P, TOT * P], BF16)
    ohT_d = ein("ohT", [P, TOT * P], BF16)
    outT = [nc.dram_tensor(f"outT{i}", [IN_DIM, NLOC], F32,
                           kind="ExternalOutput") for i in range(3)]

    with tile.TileContext(nc) as tc:
        with (
            tc.tile_pool(name="wpool", bufs=1) as wp,
            tc.tile_pool(name="dram", bufs=1, space="DRAM") as dp,
            tc.tile_pool(name="persist", bufs=1) as pp,
        ):
            # gather DMA-completion sems for prepare_only mode. Tile only
            # clears ITS OWN sems at kernel tail, so across NEFF executions
            # (warmup + profiled run) a user sem keeps stale counts and
            # consumer waits release early -> garbage reads. Clear at start.
            gsem0 = nc.alloc_semaphore("gsem0")
            gsem1 = nc.alloc_semaphore("gsem1")
            grng = range(min(gsem0.num, gsem1.num),
                         max(gsem0.num, gsem1.num) + 1)
            nc.gpsimd.dma_reset(grng)
            nc.gpsimd.sem_clear(grng)

            # ------------- internal DRAM -------------
            aspace = "Shared" if NCORE > 4 else "Local"
            ag_in0 = dp.tile([NLOC, EXT0], BF16)
            table0 = [dp.tile([SPLIT, EXT0], BF16, addr_space=aspace,
                              name=f"table0_{i}") for i in range(2)]
            ag_in1 = dp.tile([NLOC, EXT1], BF16)
            table1 = [dp.tile([SPLIT, EXT1], BF16, addr_space=aspace,
                              name=f"table1_{i}") for i in range(2)]
            r0T_d = dp.tile([Z0, NLOC], BF16)
            r1T_d = dp.tile([Z1, NLOC], BF16)

            # ------------- persistent SBUF -------------
            ident = pp.tile([P, P], BF16)
            make_identity(nc, ident[:])
            er0_all = pp.tile([P, NB * 8], BF16)
            er1_all = pp.tile([P, NB * 8], BF16)
            nc.vector.memset(er0_all[:], 0.0)
            nc.vector.memset(er1_all[:], 0.0)

            # weights to SBUF (decode weights loaded late, in the decode
            # pool, so edge-phase pools can use the space)
            def wload(name, t, kparts, pool=None):
                w = (pool or wp).tile([P, kparts, t.shape[1]], BF16, name=name)
                nc.sync.dma_start(
                    w[:], t[:, :].rearrange("(k p) m -> p k m", p=P))
                return w
            Wemb_s = [wload(f"wemb{i}", Wemb[i], KI) for i in range(3)]
            fc0_s = wload("fc0s", fc0, KC)
            res0_s = wload("res0s", res0, KC)
            alr0_s = wload("alr0s", alr0, K0)
            fc1_s = wload("fc1s", fc1, K0)
            res1_s = wload("res1s", res1, K0)
            alr1_s = wload("alr1s", alr1, K1)
            bemb_s = []
            for i in range(3):
                b = wp.tile([P, 1], F32, name=f"bemb{i}s")
                nc.sync.dma_start(b[:], bemb[i][:, :])
                bemb_s.append(b)
            bd_s = []
            for i in range(3):
                b = wp.tile([P, IN_DIM // P], F32, name=f"bd{i}s")
                nc.sync.dma_start(b[:], bd[i][:, :])
                bd_s.append(b)

            # =========================================================
            # helper: dense matmul  outT_sb[:, mb, c0:c1] over chunks
            # =========================================================
            def dense(out_sb, sbuf_pool, lhs_sb, kparts, mblocks, rhs_sb,
                      psum_pool, act_fn, bias=None, out_f32_to=None,
                      name=""):
                """out[mb][P, chunk] = act( sum_k lhs[k].T @ rhs[k] + bias )"""
                for mb in range(mblocks):
                    for (a, b) in nch:
                        w = b - a
                        ps = psum_pool.tile([P, FW], F32, tag="dps")
                        for k in range(kparts):
                            nc.tensor.matmul(
                                ps[:, :w],
                                lhsT=lhs_sb[:, k, mb * P:(mb + 1) * P],
                                rhs=rhs_sb[:, k, a:b],
                                start=(k == 0), stop=(k == kparts - 1))
                        kw = {}
                        if bias is not None:
                            kw['bias'] = bias(mb)
                        if out_f32_to is not None:
                            o = sbuf_pool.tile([P, FW], F32, tag="dout")
                            nc.scalar.activation(o[:, :w], ps[:, :w],
                                                 act_fn, **kw)
                            nc.sync.dma_start(out_f32_to[mb * P:(mb + 1) * P,
                                                         a:b], o[:, :w])
                        else:
                            nc.scalar.activation(out_sb[:, mb, a:b],
                                                 ps[:, :w], act_fn, **kw)

            # =========================================================
            # D0: embedding -> hT  (CONCAT x NLOC, bf16, SBUF)
            # =========================================================
            with tc.tile_pool(name="d0psum", bufs=2, space="PSUM") as psp, \
                 tc.tile_pool(name="d0sb", bufs=3) as sbp:
                hT = pp.tile([P, KC, NLOC], BF16, name="hT", tag="hbuf")
                for fb in range(3):
                    for (a, b) in nch:
                        w = b - a
                        ps = psp.tile([P, FW], F32, tag="emb")
                        for k in range(KI):
                            rt = sbp.tile([P, FW], BF16, tag="feat")
                            nc.sync.dma_start(
                                rt[:, :w], featT[fb][k * P:(k + 1) * P, a:b])
                            nc.tensor.matmul(ps[:, :w],
                                             lhsT=Wemb_s[fb][:, k, :],
                                             rhs=rt[:, :w],
                                             start=(k == 0),
                                             stop=(k == KI - 1))
                        nc.scalar.activation(hT[:, fb, a:b], ps[:, :w],
                                             AF.Relu, bias=bemb_s[fb][:, :1])

            # =========================================================
            # D0b: z0T, el/er, r0T, table0 assembly, AllGather
            # =========================================================
            def proj_layer(h_sb, kparts, fc_sb, res_sb, alr_sb, zparts,
                           er_all, ag_in, table, rT_d, EXT, Z):
                with tc.tile_pool(name="p1psum", bufs=2, space="PSUM") as psp, \
                     tc.tile_pool(name="p1sb", bufs=3) as sbp, \
                     tc.tile_pool(name="p1z", bufs=1) as zp:
                    zT = zp.tile([P, zparts, NLOC], BF16, name="zT")
                    # zT = fc.T @ h
                    for zb in range(zparts):
                        for (a, b) in nch:
                            w = b - a
                            ps = psp.tile([P, FW], F32, tag="z")
                            for k in range(kparts):
                                nc.tensor.matmul(
                                    ps[:, :w],
                                    lhsT=fc_sb[:, k, zb * P:(zb + 1) * P],
                                    rhs=h_sb[:, k, a:b],
                                    start=(k == 0), stop=(k == kparts - 1))
                            nc.scalar.activation(zT[:, zb, a:b], ps[:, :w],
                                                 AF.Copy)
                    # el/er (8 rows) from zT
                    lr_hi = pp.tile([8, NLOC], BF16, name="lrhi", tag="lrhi")
                    lr_lo = pp.tile([8, NLOC], BF16, name="lrlo", tag="lrlo")
                    for (a, b) in nch:
                        w = b - a
                        ps = psp.tile([8, FW], F32, tag="lr")
                        for zb in range(zparts):
                            nc.tensor.matmul(ps[:, :w],
                                             lhsT=alr_sb[:, zb, :],
                                             rhs=zT[:, zb, a:b],
                                             start=(zb == 0),
                                             stop=(zb == zparts - 1))
                        nc.vector.tensor_copy(lr_hi[:, a:b], ps[:, :w])
                        nc.vector.tensor_tensor(lr_lo[:, a:b], ps[:, :w],
                                                lr_hi[:, a:b], op=OP.subtract)
                    # assemble node-major table rows + er_all
                    for nb in range(NB):
                        a = nb * P
                        w = min(P, NLOC - a)
                        stage = sbp.tile([P, EXT], BF16, tag="stage")
                        nc.vector.memset(stage[:, Z + 8:EXT], 0.0)
                        for zb in range(zparts):
                            pt = psp.tile([P, P], BF16, tag="tr")
                            nc.tensor.transpose(pt[:w, :], zT[:, zb, a:a + w],
                                                ident[:])
                            nc.vector.tensor_copy(
                                stage[:w, zb * P:(zb + 1) * P], pt[:w, :])
                        pt = psp.tile([P, 8], BF16, tag="tr")
                        nc.tensor.transpose(pt[:w, :], lr_hi[:, a:a + w],
                                            ident[:8, :8])
                        nc.vector.tensor_copy(stage[:w, Z:Z + 4], pt[:w, 0:4])
                        nc.vector.tensor_copy(er_all[:w, nb * 8:nb * 8 + 4],
                                              pt[:w, 4:8])
                        pt2 = psp.tile([P, 8], BF16, tag="tr")
                        nc.tensor.transpose(pt2[:w, :], lr_lo[:, a:a + w],
                                            ident[:8, :8])
                        nc.vector.tensor_copy(stage[:w, Z + 4:Z + 8],
                                              pt2[:w, 0:4])
                        nc.vector.tensor_copy(
                            er_all[:w, nb * 8 + 4:nb * 8 + 8], pt2[:w, 4:8])
                        nc.sync.dma_start(ag_in[a:a + w, :], stage[:w, :])
                        if a < HALF_LOC <= a + w:
                            # first-half table chunk: lets the edge phase
                            # start on half-0 while the rest gathers
                            nc.gpsimd.collective_compute(
                                "AllGather", OP.bypass,
                                replica_groups=[list(range(NCORE))],
                                ins=[ag_in[:HALF_LOC, :]],
                                outs=[table[0][:]])
                    nc.gpsimd.collective_compute(
                        "AllGather", OP.bypass,
                        replica_groups=[list(range(NCORE))],
                        ins=[ag_in[HALF_LOC:, :]], outs=[table[1][:]])
                    # residual projection rT
                    for rb in range(zparts):
                        for (a, b) in nch:
                            w = b - a
                            ps = psp.tile([P, FW], F32, tag="r")
                            for k in range(kparts):
                                nc.tensor.matmul(
                                    ps[:, :w],
                                    lhsT=res_sb[:, k, rb * P:(rb + 1) * P],
                                    rhs=h_sb[:, k, a:b],
                                    start=(k == 0), stop=(k == kparts - 1))
                            ot = sbp.tile([P, FW], BF16, tag="rout")
                            nc.scalar.activation(ot[:, :w], ps[:, :w], AF.Copy)
                            nc.sync.dma_start(rT_d[rb * P:(rb + 1) * P, a:b],
                                              ot[:, :w])

            proj_layer(hT, KC, fc0_s, res0_s, alr0_s, K0, er0_all,
                       ag_in0, table0, r0T_d, EXT0, Z0)

            # =========================================================
            # E: edge phase (shared for both layers)
            # =========================================================
            def edge_phase(table, er_all, rT_d, h_out, relu, EXT, Z, gsem):
                zparts = Z // P
                with tc.tile_pool(name="epsum", bufs=2, space="PSUM") as psp, \
                     tc.tile_pool(name="esb", bufs=2) as sbp, \
                     tc.tile_pool(name="ezg", bufs=4) as zgp, \
                     tc.tile_pool(name="esb2", bufs=2) as sbp2:
                    # half-0 partial sums stash (bf16) so half-1 can run
                    # as a separate pass, overlapping the second AllGather
                    stash = pp.tile([P, NB, Z + 4], BF16, tag="stash")
                    for half in range(2):
                        for blk in range(NB):
                            psz = psp.tile([P, Z + 4], F32, tag="psz")
                            seg = (blk * 2 + half)
                            T = segT[seg]
                            o = offs[seg]
                            ohs = sbp2.tile([P, T * P], BF16, tag="ohs")
                            nc.sync.dma_start(
                                ohs[:], oh_d[:, o * P:(o + T) * P])
                            ohTs = sbp2.tile([P, T * P], BF16, tag="ohTs")
                            nc.sync.dma_start(
                                ohTs[:], ohT_d[:, o * P:(o + T) * P])
                            tbl = table[half][:]
                            for (t0, t1) in _chunks(T, GB):
                                nt = t1 - t0
                                sz = nt * P
                                it = sbp.tile([P, GB * 8], I16, tag="idx")
                                nc.sync.dma_start(
                                    it[:, :nt * 8],
                                    idx_d[:, (o + t0) * 8:(o + t1) * 8])
                                zg = zgp.tile([P, GB, EXT], BF16, tag="zg")
                                if USE_PREP:
                                    nc.gpsimd.dma_gather(
                                        zg[:, :nt, :], tbl, it[:, :nt * 8],
                                        sz, sz, EXT, prepare_only=True,
                                        sem=gsem)
                                    nc.gpsimd.trigger_dma(count=None)
                                else:
                                    nc.gpsimd.dma_gather(
                                        zg[:, :nt, :], tbl, it[:, :nt * 8],
                                        sz, sz, EXT)
                                per = psp.tile([P, GB, 8], F32, tag="per")
                                for ts in range(nt):
                                    nc.tensor.matmul(
                                        per[:, ts, :],
                                        lhsT=ohs_T_slice(ohTs, t0 + ts),
                                        rhs=er_all[:, blk * 8:blk * 8 + 8],
                                        start=True, stop=True)
                                # u computation (batched over nt tiles)
                                el = sbp.tile([P, GB, 4], F32, tag="el")
                                nc.vector.tensor_tensor(
                                    el[:, :nt, :], zg[:, :nt, Z:Z + 4],
                                    zg[:, :nt, Z + 4:Z + 8], op=OP.add)
                                nc.vector.tensor_tensor(
                                    el[:, :nt, :], el[:, :nt, :],
                                    per[:, :nt, 0:4], op=OP.add)
                                nc.vector.tensor_tensor(
                                    el[:, :nt, :], el[:, :nt, :],
                                    per[:, :nt, 4:8], op=OP.add)
                                nc.vector.scalar_tensor_tensor(
                                    el[:, :nt, :], el[:, :nt, :], 0.2,
                                    el[:, :nt, :], op0=OP.mult, op1=OP.max)
                                # u (bf16) lands in zg cols [Z:Z+4): the psz
                                # matmul over [:Z+4) then also accumulates
                                # sum(u) per dst in psz[:, Z:Z+4).
                                nc.scalar.activation(
                                    zg[:, :nt, Z:Z + 4], el[:, :nt, :],
                                    AF.Exp)
                                # batched per-head weighting via 0-stride
                                # bcast, in place on the gathered tile
                                hw = Z // H
                                for h in range(H):
                                    i0, i1 = broadcast_tensor_aps(
                                        zg[:, :nt, h * hw:(h + 1) * hw],
                                        zg[:, :nt, Z + h:Z + h + 1])
                                    nc.vector.tensor_tensor(
                                        zg[:, :nt, h * hw:(h + 1) * hw],
                                        i0, i1, op=OP.mult)
                                for ts in range(nt):
                                    t = t0 + ts
                                    first = (t == 0)
                                    last = (t == T - 1)
                                    if Z + 4 <= 512:
                                        nc.tensor.matmul(
                                            psz[:],
                                            lhsT=ohs[:, t * P:(t + 1) * P],
                                            rhs=zg[:, ts, :Z + 4],
                                            start=first, stop=last)
                                    else:
                                        # matmul free dim caps at 512
                                        nc.tensor.matmul(
                                            psz[:, :Z],
                                            lhsT=ohs[:, t * P:(t + 1) * P],
                                            rhs=zg[:, ts, :Z],
                                            start=first, stop=last)
                                        nc.tensor.matmul(
                                            psz[:, Z:Z + 4],
                                            lhsT=ohs[:, t * P:(t + 1) * P],
                                            rhs=zg[:, ts, Z:Z + 4],
                                            start=first, stop=last)
                            if half == 0:
                                # stash half-0 partials (bf16)
                                nc.vector.tensor_copy(stash[:, blk, :],
                                                      psz[:])
                                continue
                            # combine halves, normalize + write back
                            tot = sbp.tile([P, Z + 4], F32, tag="tot")
                            nc.vector.tensor_tensor(tot[:], psz[:],
                                                    stash[:, blk, :],
                                                    op=OP.add)
                            sp = sbp.tile([P, 4], F32, tag="sp")
                            nc.vector.tensor_scalar_add(sp[:],
                                                        tot[:, Z:Z + 4],
                                                        1e-9)
                            rp = sbp.tile([P, 4], F32, tag="rp")
                            nc.vector.reciprocal(rp[:], sp[:])
                            rt = sbp.tile([P, Z], BF16, tag="rstt")
                            hw = Z // H
                            for h in range(H):
                                nc.vector.tensor_scalar_mul(
                                    rt[:, h * hw:(h + 1) * hw],
                                    tot[:, h * hw:(h + 1) * hw],
                                    rp[:, h:h + 1])
                            # fused untranspose: write this block straight
                            # into the next phase's [Z, NLOC] activation so
                            # downstream column-chunks can start while later
                            # blocks are still aggregating
                            a = blk * P
                            w = min(P, NLOC - a)
                            for zb in range(zparts):
                                pt = psp.tile([P, P], BF16, tag="trp")
                                nc.tensor.transpose(
                                    pt[:, :w], rt[:w, zb * P:(zb + 1) * P],
                                    ident[:w, :w])
                                rr = sbp.tile([P, P], BF16, tag="rr")
                                nc.sync.dma_start(
                                    rr[:, :w],
                                    rT_d[zb * P:(zb + 1) * P, a:a + w])
                                nc.vector.tensor_tensor(
                                    h_out[:, zb, a:a + w], pt[:, :w],
                                    rr[:, :w], op=OP.add)
                                if relu:
                                    nc.scalar.activation(
                                        h_out[:, zb, a:a + w],
                                        h_out[:, zb, a:a + w], AF.Relu)

            def ohs_T_slice(ohTs, t):
                return ohTs[:, t * P:(t + 1) * P]

            h1T = pp.tile([P, K0, NLOC], BF16, name="h1T", tag="hbuf")
            edge_phase(table0, er0_all, r0T_d, h1T, True, EXT0, Z0, gsem0)

            # ============== D1 + E1 ==============
            proj_layer(h1T, K0, fc1_s, res1_s, alr1_s, K1, er1_all,
                       ag_in1, table1, r1T_d, EXT1, Z1)
            d0T = pp.tile([P, K1, NLOC], BF16, name="d0T", tag="hbuf")
            edge_phase(table1, er1_all, r1T_d, d0T, False, EXT1, Z1, gsem1)

            # ============== decode ==============
            with tc.tile_pool(name="decp", bufs=2, space="PSUM") as psp, \
                 tc.tile_pool(name="decs", bufs=3) as sbp, \
                 tc.tile_pool(name="dwp", bufs=1) as dwp:
                fc1T_s = wload("fc1Ts", fc1T, K1, pool=dwp)
                fc0T_s = wload("fc0Ts", fc0T, K0, pool=dwp)
                WdT4_s = [wload(f"wdt{i}", WdT4[i], KC, pool=dwp)
                          for i in range(3)]
                # relu on d0T in-place
                for zb in range(K1):
                    for (a, b) in nch:
                        nc.scalar.activation(d0T[:, zb, a:b], d0T[:, zb, a:b],
                                             AF.Relu)
                d1T = dwp.tile([P, K0, NLOC], BF16, name="d1T")
                dense(d1T, sbp, fc1T_s, K1, K0, d0T, psp, AF.Relu, name="d1")
                d2T = pp.tile([P, KC, NLOC], BF16, name="d2T", tag="hbuf")
                dense(d2T, sbp, fc0T_s, K0, KC, d1T, psp, AF.Relu, name="d2")
                for i in range(3):
                    dense(None, sbp, WdT4_s[i], KC, IN_DIM // P, d2T, psp,
                          AF.Sigmoid, bias=lambda mb, i=i: bd_s[i][:, mb:mb + 1],
                          out_f32_to=outT[i], name=f"o{i}")

    nc.compile()
    return nc


# =====================================================================
# Host side
# =====================================================================

def _host_prep(inputs, cfg):
    N, NCORE, NLOC, NB = cfg['N'], cfg['NCORE'], cfg['NLOC'], cfg['NB']
    SPLIT, H = cfg['SPLIT'], cfg['H']
    bf = ml_dtypes.bfloat16
    src = np.asarray(inputs['src']); dst = np.asarray(inputs['dst'])
    core = dst // NLOC
    dloc = dst % NLOC
    blk = dloc // P
    # table halves are chunked by LOCAL row (AllGather chunk = first/second
    # half of each core's rows): table row = owner*HALF_LOC + local_off
    HALF_LOC = NLOC // 2
    half = ((src % NLOC) >= HALF_LOC).astype(np.int64)
    # tile count per (core, blk, half); per-segment tile counts are the max
    # over cores so the single SPMD program fits every core's stream.
    cnt = np.zeros((NCORE, NB, 2), np.int64)
    np.add.at(cnt, (core, blk, half), 1)
    segT = np.maximum(1, -(-cnt.max(axis=0) // P)).reshape(-1)  # [NB*2]
    offs = np.concatenate([[0], np.cumsum(segT)])               # tile offsets
    TOT = int(offs[-1])

    # shared (per-core identical) weights
    sh = {}
    for i in range(3):
        sh[f'Wemb{i}'] = np.ascontiguousarray(inputs[f'W_emb{i}']).astype(bf)
        sh[f'bemb{i}'] = np.asarray(inputs[f'b_emb{i}'],
                                    np.float32).reshape(-1, 1)
        wd = np.asarray(inputs[f'Wd{i}'], np.float32)
        sh[f'WdT4{i}'] = np.ascontiguousarray(
            np.concatenate([wd] * H, axis=0) * (1.0 / H)).astype(bf)
        sh[f'bd{i}'] = np.ascontiguousarray(
            np.asarray(inputs[f'bd{i}'], np.float32).reshape(-1, P).T)
    sh['fc0'] = np.asarray(inputs['fc0']).astype(bf)
    sh['res0'] = np.asarray(inputs['res0']).astype(bf)
    sh['fc1'] = np.asarray(inputs['fc1']).astype(bf)
    sh['res1'] = np.asarray(inputs['res1']).astype(bf)
    sh['fc1T'] = np.ascontiguousarray(np.asarray(inputs['fc1']).T).astype(bf)
    sh['fc0T'] = np.ascontiguousarray(np.asarray(inputs['fc0']).T).astype(bf)
    for li in (0, 1):
        al = np.asarray(inputs[f'al{li}'], np.float32)
        ar = np.asarray(inputs[f'ar{li}'], np.float32)
        Hh, D = al.shape
        blkm = np.zeros((Hh * D, 8), np.float32)
        for h in range(Hh):
            blkm[h * D:(h + 1) * D, h] = al[h]
            blkm[h * D:(h + 1) * D, 4 + h] = ar[h]
        sh[f'alr{li}'] = blkm.astype(bf)

    # per-core edge streams + one-hots
    per_core = []
    order_all = np.lexsort((dloc, half, blk, core))
    src_s = src[order_all]; dloc_s = dloc[order_all]
    blk_s = blk[order_all]; half_s = half[order_all]; core_s = core[order_all]
    core_off = np.searchsorted(core_s, np.arange(NCORE + 1))
    for c in range(NCORE):
        s0, s1 = core_off[c], core_off[c + 1]
        es, ed, eb, eh = (src_s[s0:s1], dloc_s[s0:s1], blk_s[s0:s1],
                          half_s[s0:s1])
        seg_id = eb * 2 + eh
        # position within each (blk, half) group
        grp_start = np.searchsorted(seg_id, np.arange(NB * 2 + 1))
        pos = np.arange(len(es)) - grp_start[seg_id]
        spos = offs[seg_id] * P + pos
        total = TOT * P
        idx16 = np.zeros(total, np.int16)
        localidx = ((es // NLOC) * HALF_LOC
                    + (es % NLOC) % HALF_LOC).astype(np.int16)
        idx16[spos] = localidx
        # wrapped idx layout per (blk,half): [16, T*8] tiled to 128 rows
        idx_arr = np.zeros((P, TOT * 8), np.int16)
        for g in range(NB * 2):
            Tg = int(segT[g])
            w = idx16[offs[g] * P:offs[g + 1] * P].reshape(Tg * 8, 16).T
            idx_arr[:, offs[g] * 8:offs[g + 1] * 8] = np.tile(w, (8, 1))
        # one-hots
        oh = np.zeros((P, total), bf)
        ohT = np.zeros((P, total), bf)
        pp_ = spos
        t_of = pp_ // P
        e_of = pp_ % P
        dr = (ed - eb * P)
        oh[e_of, t_of * P + dr] = 1
        ohT[dr, t_of * P + e_of] = 1
        d = {'idx': idx_arr, 'oh': oh, 'ohT': ohT}
        r0, r1 = c * NLOC, (c + 1) * NLOC
        for i in range(3):
            d[f'featT{i}'] = np.ascontiguousarray(
                np.asarray(inputs[f'feat{i}'])[r0:r1].T).astype(bf)
        per_core.append(d)
    return sh, per_core, tuple(int(t) for t in segT)


_CACHE = {}


def _run(inputs, **kw):
    cfg = CFG
    sh, per_core, segT = _host_prep(inputs, cfg)
    key = ('v2', segT)
    if key not in _CACHE:
        _CACHE[key] = build_bass(cfg, segT)
    nc = _CACHE[key]
    in_maps = [{**sh, **pc} for pc in per_core]
    res = run_bass_kernel_spmd(nc, in_maps,
                               core_ids=list(range(cfg['NCORE'])), **kw)
    outs = []
    for i in range(3):
        outs.append(np.concatenate(
            [np.asarray(res.results[c][f'outT{i}'], np.float32).T
             for c in range(cfg['NCORE'])], axis=0))
    return tuple(outs), res


def kernel(**inputs):
    outs, _ = _run(inputs)
    return outs



# revision 44
# speedup vs baseline: 1.3141x; 1.2158x over previous
"""Trainium2 Bass kernel for CancerGATE (3-omics GAT autoencoder).

Sharding: nodes row-sharded across 8 NeuronCores. Dense phases (embedding,
projections, decode) run on each core's 6250-node shard in a transposed
layout (features on partitions, nodes on the free dim). The projected
features + attention-left logits are AllGathered into two per-core DRAM
table halves (chunked by local row so the second chunk's collective
overlaps edge processing of the first); the edge phase gathers source rows
by edge (dma_gather, int16 indices), weights them by the attention
coefficient (bf16 u written into the row's spare columns so one one-hot
matmul accumulates both the weighted-z sum and the softmax denominator)
and scatter-adds into per-destination-block PSUM via host-built one-hot
matmuls. The edge loop runs half-outer with a bf16 SBUF stash of half-0
partials. Per-(dst-block, half) edge-tile counts are the max over cores,
baked into the program (~6% less padding than a global max). Edge softmax
uses the unnormalized form (exp without max subtraction -- the logit range
for this model is [-3, 4]) so normalization is a single reciprocal per
destination node after aggregation.
"""
import sys
sys.path.insert(0, '/opt/trn_rl_repo')

import numpy as np
import ml_dtypes

import concourse.bass as bass
import concourse.bacc as bacc
import concourse.tile as tile
from concourse import mybir
from concourse.bass import IndirectOffsetOnAxis, broadcast_tensor_aps
from concourse.bass_utils import run_bass_kernel_spmd
from concourse.masks import make_identity

USE_INDIRECT = False  # HW layout of indirect gather differs from sim; using dma_gather
USE_PREP = False  # prepare_only+trigger races on HW even with cleared sems

F32 = mybir.dt.float32
BF16 = mybir.dt.bfloat16
I16 = mybir.dt.int16
AF = mybir.ActivationFunctionType
OP = mybir.AluOpType

P = 128
GB = 8  # tiles per dma_gather batch (1024 indices; >=1536 wedges the device)


def _dcfg(N=50000, NCORE=8, IN_DIM=512, D0=128, H=4, O0=128, O1=64, FW=512):
    c = {}
    c['N'] = N; c['NCORE'] = NCORE; c['IN_DIM'] = IN_DIM; c['D0'] = D0
    c['H'] = H; c['O0'] = O0; c['O1'] = O1
    c['CONCAT'] = 3 * D0
    c['Z0'] = H * O0
    c['Z1'] = H * O1
    c['DEC'] = c['CONCAT'] // H
    c['NLOC'] = N // NCORE
    c['NB'] = -(-c['NLOC'] // P)
    c['SPLIT'] = N // 2
    # table row widths (bf16 cols): z + 4 el_hi + 4 el_lo, padded to 128 cols
    c['EXT0'] = -(-(c['Z0'] + 8) // P) * P
    c['EXT1'] = -(-(c['Z1'] + 8) // P) * P
    c['FW'] = FW
    return c


CFG = _dcfg()


def _chunks(total, w):
    return [(a, min(a + w, total)) for a in range(0, total, w)]


def _f2(ap):
    """Flatten a sliced 3D AP to 2D [P, cols]."""
    return ap.rearrange("p a b -> p (a b)")


def build_bass(cfg, segT):
    N, NCORE, NLOC, NB = cfg['N'], cfg['NCORE'], cfg['NLOC'], cfg['NB']
    IN_DIM, CONCAT, Z0, Z1, DEC = (cfg['IN_DIM'], cfg['CONCAT'], cfg['Z0'],
                                   cfg['Z1'], cfg['DEC'])
    EXT0, EXT1, H, FW = cfg['EXT0'], cfg['EXT1'], cfg['H'], cfg['FW']
    SPLIT = cfg['SPLIT']
    KI = IN_DIM // P      # k-chunks for IN_DIM contraction
    KC = CONCAT // P      # k-chunks for CONCAT
    K0 = Z0 // P          # z0 partition blocks
    K1 = Z1 // P          # z1 partition blocks
    KD = -(-DEC * H // P) # = KC
    LW = NLOC - (NB - 1) * P  # last node-block width
    HALF_LOC = NLOC // 2      # AllGather chunk boundary (local rows)
    nch = _chunks(NLOC, cfg['FW'])
    # per-(blk,half) segment tile counts (max over cores, host-computed)
    segT = list(segT)
    offs = [0]
    for t in segT:
        offs.append(offs[-1] + t)
    TOT = offs[-1]        # total edge tiles per core

    nc = bacc.Bacc("TRN2", target_bir_lowering=False, debug=False,
                   num_devices=NCORE)

    # ---------------- I/O ----------------
    ein = lambda nm, sh, dt: nc.dram_tensor(nm, sh, dt, kind="ExternalInput")
    featT = [ein(f"featT{i}", [IN_DIM, NLOC], BF16) for i in range(3)]
    Wemb = [ein(f"Wemb{i}", [IN_DIM, cfg['D0']], BF16) for i in range(3)]
    bemb = [ein(f"bemb{i}", [cfg['D0'], 1], F32) for i in range(3)]
    fc0 = ein("fc0", [CONCAT, Z0], BF16)
    res0 = ein("res0", [CONCAT, Z0], BF16)
    alr0 = ein("alr0", [Z0, 8], BF16)
    fc1 = ein("fc1", [Z0, Z1], BF16)
    res1 = ein("res1", [Z0, Z1], BF16)
    alr1 = ein("alr1", [Z1, 8], BF16)
    fc1T = ein("fc1T", [Z1, Z0], BF16)
    fc0T = ein("fc0T", [Z0, CONCAT], BF16)
    WdT4 = [ein(f"WdT4{i}", [CONCAT, IN_DIM], BF16) for i in range(3)]
    bd = [ein(f"bd{i}", [P, IN_DIM // P], F32) for i in range(3)]
    idx_d = ein("idx", [P, TOT * 8], I16)
    oh_d = ein("oh", [P, TOT * P], BF16)
    ohT_d = ein("ohT", [P, TOT * P], BF16)
    outT = [nc.dram_tensor(f"outT{i}", [IN_DIM, NLOC], F32,
                           kind="ExternalOutput") for i in range(3)]

    with tile.TileContext(nc) as tc:
        with (
            tc.tile_pool(name="wpool", bufs=1) as wp,
            tc.tile_pool(name="dram", bufs=1, space="DRAM") as dp,
            tc.tile_pool(name="persist", bufs=1) as pp,
        ):
            # gather DMA-completion sems for prepare_only mode. Tile only
            # clears ITS OWN sems at kernel tail, so across NEFF executions
            # (warmup + profiled run) a user sem keeps stale counts and
            # consumer waits release early -> garbage reads. Clear at start.
            gsem0 = nc.alloc_semaphore("gsem0")
            gsem1 = nc.alloc_semaphore("gsem1")
            grng = range(min(gsem0.num, gsem1.num),
                         max(gsem0.num, gsem1.num) + 1)
            nc.gpsimd.dma_reset(grng)
            nc.gpsimd.sem_clear(grng)

            # ------------- internal DRAM -------------
            aspace = "Shared" if NCORE > 4 else "Local"
            ag_in0 = dp.tile([NLOC, EXT0], BF16)
            table0 = [dp.tile([SPLIT, EXT0], BF16, addr_space=aspace,
                              name=f"table0_{i}") for i in range(2)]
            ag_in1 = dp.tile([NLOC, EXT1], BF16)
            table1 = [dp.tile([SPLIT, EXT1], BF16, addr_space=aspace,
                              name=f"table1_{i}") for i in range(2)]
            r0T_d = dp.tile([Z0, NLOC], BF16)
            r1T_d = dp.tile([Z1, NLOC], BF16)

            # ------------- persistent SBUF -------------
            ident = pp.tile([P, P], BF16)
            make_identity(nc, ident[:])
            er0_all = pp.tile([P, NB * 8], BF16)
            er1_all = pp.tile([P, NB * 8], BF16)
            nc.vector.memset(er0_all[:], 0.0)
            nc.vector.memset(er1_all[:], 0.0)

            # weights to SBUF (decode weights loaded late, in the decode
            # pool, so edge-phase pools can use the space)
            def wload(name, t, kparts, pool=None):
                w = (pool or wp).tile([P, kparts, t.shape[1]], BF16, name=name)
                nc.sync.dma_start(
                    w[:], t[:, :].rearrange("(k p) m -> p k m", p=P))
                return w
            Wemb_s = [wload(f"wemb{i}", Wemb[i], KI) for i in range(3)]
            fc0_s = wload("fc0s", fc0, KC)
            res0_s = wload("res0s", res0, KC)
            alr0_s = wload("alr0s", alr0, K0)
            fc1_s = wload("fc1s", fc1, K0)
            res1_s = wload("res1s", res1, K0)
            alr1_s = wload("alr1s", alr1, K1)
            bemb_s = []
            for i in range(3):
                b = wp.tile([P, 1], F32, name=f"bemb{i}s")
                nc.sync.dma_start(b[:], bemb[i][:, :])
                bemb_s.append(b)
            bd_s = []
            for i in range(3):
                b = wp.tile([P, IN_DIM // P], F32, name=f"bd{i}s")
                nc.sync.dma_start(b[:], bd[i][:, :])
                bd_s.append(b)

            # =========================================================
            # helper: dense matmul  outT_sb[:, mb, c0:c1] over chunks
            # =========================================================
            def dense(out_sb, sbuf_pool, lhs_sb, kparts, mblocks, rhs_sb,
                      psum_pool, act_fn, bias=None, out_f32_to=None,
                      name=""):
                """out[mb][P, chunk] = act( sum_k lhs[k].T @ rhs[k] + bias )"""
                for mb in range(mblocks):
                    for (a, b) in nch:
                        w = b - a
                        ps = psum_pool.tile([P, FW], F32, tag="dps")
                        for k in range(kparts):
                            nc.tensor.matmul(
                                ps[:, :w],
                                lhsT=lhs_sb[:, k, mb * P:(mb + 1) * P],
                                rhs=rhs_sb[:, k, a:b],
                                start=(k == 0), stop=(k == kparts - 1))
                        kw = {}
                        if bias is not None:
                            kw['bias'] = bias(mb)
                        if out_f32_to is not None:
                            o = sbuf_pool.tile([P, FW], F32, tag="dout")
                            nc.scalar.activation(o[:, :w], ps[:, :w],
                                                 act_fn, **kw)
                            nc.sync.dma_start(out_f32_to[mb * P:(mb + 1) * P,
                                                         a:b], o[:, :w])
                        else:
                            nc.scalar.activation(out_sb[:, mb, a:b],
                                                 ps[:, :w], act_fn, **kw)

            # =========================================================
            # D0: embedding -> hT  (CONCAT x NLOC, bf16, SBUF)
            # =========================================================
            with tc.tile_pool(name="d0psum", bufs=2, space="PSUM") as psp, \
                 tc.tile_pool(name="d0sb", bufs=3) as sbp:
                hT = pp.tile([P, KC, NLOC], BF16, name="hT", tag="hbuf")
                for fb in range(3):
                    for (a, b) in nch:
                        w = b - a
                        ps = psp.tile([P, FW], F32, tag="emb")
                        for k in range(KI):
                            rt = sbp.tile([P, FW], BF16, tag="feat")
                            nc.sync.dma_start(
                                rt[:, :w], featT[fb][k * P:(k + 1) * P, a:b])
                            nc.tensor.matmul(ps[:, :w],
                                             lhsT=Wemb_s[fb][:, k, :],
                                             rhs=rt[:, :w],
                                             start=(k == 0),
                                             stop=(k == KI - 1))
                        nc.scalar.activation(hT[:, fb, a:b], ps[:, :w],
                                             AF.Relu, bias=bemb_s[fb][:, :1])

            # =========================================================
            # D0b: z0T, el/er, r0T, table0 assembly, AllGather
            # =========================================================
            def proj_layer(h_sb, kparts, fc_sb, res_sb, alr_sb, zparts,
                           er_all, ag_in, table, rT_d, EXT, Z):
                with tc.tile_pool(name="p1psum", bufs=2, space="PSUM") as psp, \
                     tc.tile_pool(name="p1sb", bufs=3) as sbp, \
                     tc.tile_pool(name="p1z", bufs=1) as zp:
                    zT = zp.tile([P, zparts, NLOC], BF16, name="zT")
                    # zT = fc.T @ h
                    for zb in range(zparts):
                        for (a, b) in nch:
                            w = b - a
                            ps = psp.tile([P, FW], F32, tag="z")
                            for k in range(kparts):
                                nc.tensor.matmul(
                                    ps[:, :w],
                                    lhsT=fc_sb[:, k, zb * P:(zb + 1) * P],
                                    rhs=h_sb[:, k, a:b],
                                    start=(k == 0), stop=(k == kparts - 1))
                            nc.scalar.activation(zT[:, zb, a:b], ps[:, :w],
                                                 AF.Copy)
                    # el/er (8 rows) from zT
                    lr_hi = pp.tile([8, NLOC], BF16, name="lrhi", tag="lrhi")
                    lr_lo = pp.tile([8, NLOC], BF16, name="lrlo", tag="lrlo")
                    for (a, b) in nch:
                        w = b - a
                        ps = psp.tile([8, FW], F32, tag="lr")
                        for zb in range(zparts):
                            nc.tensor.matmul(ps[:, :w],
                                             lhsT=alr_sb[:, zb, :],
                                             rhs=zT[:, zb, a:b],
                                             start=(zb == 0),
                                             stop=(zb == zparts - 1))
                        nc.vector.tensor_copy(lr_hi[:, a:b], ps[:, :w])
                        nc.vector.tensor_tensor(lr_lo[:, a:b], ps[:, :w],
                                                lr_hi[:, a:b], op=OP.subtract)
                    # assemble node-major table rows + er_all
                    for nb in range(NB):
                        a = nb * P
                        w = min(P, NLOC - a)
                        stage = sbp.tile([P, EXT], BF16, tag="stage")
                        nc.vector.memset(stage[:, Z + 8:EXT], 0.0)
                        for zb in range(zparts):
                            pt = psp.tile([P, P], BF16, tag="tr")
                            nc.tensor.transpose(pt[:w, :], zT[:, zb, a:a + w],
                                                ident[:])
                            nc.vector.tensor_copy(
                                stage[:w, zb * P:(zb + 1) * P], pt[:w, :])
                        pt = psp.tile([P, 8], BF16, tag="tr")
                        nc.tensor.transpose(pt[:w, :], lr_hi[:, a:a + w],
                                            ident[:8, :8])
                        nc.vector.tensor_copy(stage[:w, Z:Z + 4], pt[:w, 0:4])
                        nc.vector.tensor_copy(er_all[:w, nb * 8:nb * 8 + 4],
                                              pt[:w, 4:8])
                        pt2 = psp.tile([P, 8], BF16, tag="tr")
                        nc.tensor.transpose(pt2[:w, :], lr_lo[:, a:a + w],
                                            ident[:8, :8])
                        nc.vector.tensor_copy(stage[:w, Z + 4:Z + 8],
                                              pt2[:w, 0:4])
                        nc.vector.tensor_copy(
                            er_all[:w, nb * 8 + 4:nb * 8 + 8], pt2[:w, 4:8])
                        nc.sync.dma_start(ag_in[a:a + w, :], stage[:w, :])
                        if a < HALF_LOC <= a + w:
                            # first-half table chunk: lets the edge phase
                            # start on half-0 while the rest gathers
                            nc.gpsimd.collective_compute(
                                "AllGather", OP.bypass,
                                replica_groups=[list(range(NCORE))],
                                ins=[ag_in[:HALF_LOC, :]],
                                outs=[table[0][:]])
                    nc.gpsimd.collective_compute(
                        "AllGather", OP.bypass,
                        replica_groups=[list(range(NCORE))],
                        ins=[ag_in[HALF_LOC:, :]], outs=[table[1][:]])
                    # residual projection rT
                    for rb in range(zparts):
                        for (a, b) in nch:
                            w = b - a
                            ps = psp.tile([P, FW], F32, tag="r")
                            for k in range(kparts):
                                nc.tensor.matmul(
                                    ps[:, :w],
                                    lhsT=res_sb[:, k, rb * P:(rb + 1) * P],
                                    rhs=h_sb[:, k, a:b],
                                    start=(k == 0), stop=(k == kparts - 1))
                            ot = sbp.tile([P, FW], BF16, tag="rout")
                            nc.scalar.activation(ot[:, :w], ps[:, :w], AF.Copy)
                            nc.sync.dma_start(rT_d[rb * P:(rb + 1) * P, a:b],
                                              ot[:, :w])

            proj_layer(hT, KC, fc0_s, res0_s, alr0_s, K0, er0_all,
                       ag_in0, table0, r0T_d, EXT0, Z0)

            # =========================================================
            # E: edge phase (shared for both layers)
            # =========================================================
            def edge_phase(table, er_all, rT_d, h_out, relu, EXT, Z, gsem):
                zparts = Z // P
                with tc.tile_pool(name="epsum", bufs=2, space="PSUM") as psp, \
                     tc.tile_pool(name="esb", bufs=2) as sbp, \
                     tc.tile_pool(name="ezg", bufs=4) as zgp, \
                     tc.tile_pool(name="esb2", bufs=2) as sbp2:
                    # half-0 partial sums stash (bf16) so half-1 can run
                    # as a separate pass, overlapping the second AllGather
                    stash = pp.tile([P, NB, Z + 4], BF16, tag="stash")

                    def finish_blk(blk, half, psz):
                        """Stash (half 0) or combine+normalize+fused
                        untranspose (half 1) once a dst block's last edge
                        tile has been accumulated."""
                        if half == 0:
                            nc.vector.tensor_copy(stash[:, blk, :], psz[:])
                            return
                        tot = sbp.tile([P, Z + 4], F32, tag="tot")
                        nc.vector.tensor_tensor(tot[:], psz[:],
                                                stash[:, blk, :], op=OP.add)
                        sp = sbp.tile([P, 4], F32, tag="sp")
                        nc.vector.tensor_scalar_add(sp[:], tot[:, Z:Z + 4],
                                                    1e-9)
                        rp = sbp.tile([P, 4], F32, tag="rp")
                        nc.vector.reciprocal(rp[:], sp[:])
                        rt = sbp.tile([P, Z], BF16, tag="rstt")
                        hw = Z // H
                        for h in range(H):
                            nc.vector.tensor_scalar_mul(
                                rt[:, h * hw:(h + 1) * hw],
                                tot[:, h * hw:(h + 1) * hw],
                                rp[:, h:h + 1])
                        # fused untranspose: write this block straight into
                        # the next phase's [Z, NLOC] activation so downstream
                        # column-chunks can start while later blocks are
                        # still aggregating
                        a = blk * P
                        w = min(P, NLOC - a)
                        for zb in range(zparts):
                            pt = psp.tile([P, P], BF16, tag="trp")
                            nc.tensor.transpose(
                                pt[:, :w], rt[:w, zb * P:(zb + 1) * P],
                                ident[:w, :w])
                            rr = sbp.tile([P, P], BF16, tag="rr")
                            nc.sync.dma_start(
                                rr[:, :w], rT_d[zb * P:(zb + 1) * P, a:a + w])
                            nc.vector.tensor_tensor(
                                h_out[:, zb, a:a + w], pt[:, :w],
                                rr[:, :w], op=OP.add)
                            if relu:
                                nc.scalar.activation(
                                    h_out[:, zb, a:a + w],
                                    h_out[:, zb, a:a + w], AF.Relu)

                    for half in range(2):
                        # gather batches stream the whole half contiguously,
                        # ignoring dst-block boundaries (fewer, fuller
                        # dma_gather calls); psz accumulators start/stop at
                        # block transitions via the per-tile block map.
                        base = half * NB
                        blk_first = {}
                        blk_last = {}
                        tile_blk = []
                        g = 0
                        for blk in range(NB):
                            Tb = segT[base + blk]
                            blk_first[blk] = g
                            g += Tb
                            blk_last[blk] = g - 1
                            tile_blk += [blk] * Tb
                        THH = g
                        o = offs[base]
                        tbl = table[half][:]
                        psz_of = {}
                        for (t0, t1) in _chunks(THH, GB):
                            nt = t1 - t0
                            sz = nt * P
                            ohs = sbp2.tile([P, GB * P], BF16, tag="ohs")
                            nc.sync.dma_start(
                                ohs[:, :nt * P],
                                oh_d[:, (o + t0) * P:(o + t1) * P])
                            ohTs = sbp2.tile([P, GB * P], BF16, tag="ohTs")
                            nc.sync.dma_start(
                                ohTs[:, :nt * P],
                                ohT_d[:, (o + t0) * P:(o + t1) * P])
                            it = sbp.tile([P, GB * 8], I16, tag="idx")
                            nc.sync.dma_start(
                                it[:, :nt * 8],
                                idx_d[:, (o + t0) * 8:(o + t1) * 8])
                            zg = zgp.tile([P, GB, EXT], BF16, tag="zg")
                            nc.gpsimd.dma_gather(
                                zg[:, :nt, :], tbl, it[:, :nt * 8],
                                sz, sz, EXT)
                            per = psp.tile([P, GB, 8], F32, tag="per")
                            for ts in range(nt):
                                b = tile_blk[t0 + ts]
                                nc.tensor.matmul(
                                    per[:, ts, :],
                                    lhsT=ohTs[:, ts * P:(ts + 1) * P],
                                    rhs=er_all[:, b * 8:b * 8 + 8],
                                    start=True, stop=True)
                            # u computation (batched over nt tiles)
                            el = sbp.tile([P, GB, 4], F32, tag="el")
                            nc.vector.tensor_tensor(
                                el[:, :nt, :], zg[:, :nt, Z:Z + 4],
                                zg[:, :nt, Z + 4:Z + 8], op=OP.add)
                            nc.vector.tensor_tensor(
                                el[:, :nt, :], el[:, :nt, :],
                                per[:, :nt, 0:4], op=OP.add)
                            nc.vector.tensor_tensor(
                                el[:, :nt, :], el[:, :nt, :],
                                per[:, :nt, 4:8], op=OP.add)
                            nc.vector.scalar_tensor_tensor(
                                el[:, :nt, :], el[:, :nt, :], 0.2,
                                el[:, :nt, :], op0=OP.mult, op1=OP.max)
                            # u (bf16) lands in zg cols [Z:Z+4): the psz
                            # matmul over [:Z+4) then also accumulates
                            # sum(u) per dst in psz[:, Z:Z+4).
                            nc.scalar.activation(
                                zg[:, :nt, Z:Z + 4], el[:, :nt, :], AF.Exp)
                            # batched per-head weighting via 0-stride bcast,
                            # in place on the gathered tile
                            hw = Z // H
                            for h in range(H):
                                i0, i1 = broadcast_tensor_aps(
                                    zg[:, :nt, h * hw:(h + 1) * hw],
                                    zg[:, :nt, Z + h:Z + h + 1])
                                nc.vector.tensor_tensor(
                                    zg[:, :nt, h * hw:(h + 1) * hw],
                                    i0, i1, op=OP.mult)
                            for ts in range(nt):
                                gi = t0 + ts
                                b = tile_blk[gi]
                                if gi == blk_first[b]:
                                    psz_of[b] = psp.tile(
                                        [P, Z + 4], F32, tag="psz",
                                        name=f"psz_h{half}_b{b}")
                                psz = psz_of[b]
                                first = (gi == blk_first[b])
                                last = (gi == blk_last[b])
                                if Z + 4 <= 512:
                                    nc.tensor.matmul(
                                        psz[:],
                                        lhsT=ohs[:, ts * P:(ts + 1) * P],
                                        rhs=zg[:, ts, :Z + 4],
                                        start=first, stop=last)
                                else:
                                    # matmul free dim caps at 512
                                    nc.tensor.matmul(
                                        psz[:, :Z],
                                        lhsT=ohs[:, ts * P:(ts + 1) * P],
                                        rhs=zg[:, ts, :Z],
                                        start=first, stop=last)
                                    nc.tensor.matmul(
                                        psz[:, Z:Z + 4],
                                        lhsT=ohs[:, ts * P:(ts + 1) * P],
                                        rhs=zg[:, ts, Z:Z + 4],
                                        start=first, stop=last)
                                if last:
                                    finish_blk(b, half, psz)
                                    del psz_of[b]

            h1T = pp.tile([P, K0, NLOC], BF16, name="h1T", tag="hbuf")
            edge_phase(table0, er0_all, r0T_d, h1T, True, EXT0, Z0, gsem0)

            # ============== D1 + E1 ==============
            proj_layer(h1T, K0, fc1_s, res1_s, alr1_s, K1, er1_all,
                       ag_in1, table1, r1T_d, EXT1, Z1)
            d0T = pp.tile([P, K1, NLOC], BF16, name="d0T", tag="hbuf")
            edge_phase(table1, er1_all, r1T_d, d0T, False, EXT1, Z1, gsem1)

            # ============== decode ==============
            with tc.tile_pool(name="decp", bufs=2, space="PSUM") as psp, \
                 tc.tile_pool(name="decs", bufs=3) as sbp, \
                 tc.tile_pool(name="dwp", bufs=1) as dwp:
                fc1T_s = wload("fc1Ts", fc1T, K1, pool=dwp)
                fc0T_s = wload("fc0Ts", fc0T, K0, pool=dwp)
                WdT4_s = [wload(f"wdt{i}", WdT4[i], KC, pool=dwp)
                          for i in range(3)]
                # relu on d0T in-place
                for zb in range(K1):
                    for (a, b) in nch:
                        nc.scalar.activation(d0T[:, zb, a:b], d0T[:, zb, a:b],
                                             AF.Relu)
                d1T = dwp.tile([P, K0, NLOC], BF16, name="d1T")
                dense(d1T, sbp, fc1T_s, K1, K0, d0T, psp, AF.Relu, name="d1")
                d2T = pp.tile([P, KC, NLOC], BF16, name="d2T", tag="hbuf")
                dense(d2T, sbp, fc0T_s, K0, KC, d1T, psp, AF.Relu, name="d2")
                for i in range(3):
                    dense(None, sbp, WdT4_s[i], KC, IN_DIM // P, d2T, psp,
                          AF.Sigmoid, bias=lambda mb, i=i: bd_s[i][:, mb:mb + 1],
                          out_f32_to=outT[i], name=f"o{i}")

    nc.compile()
    return nc


# =====================================================================
# Host side
# =====================================================================

def _host_prep(inputs, cfg):
    N, NCORE, NLOC, NB = cfg['N'], cfg['NCORE'], cfg['NLOC'], cfg['NB']
    SPLIT, H = cfg['SPLIT'], cfg['H']
    bf = ml_dtypes.bfloat16
    src = np.asarray(inputs['src']); dst = np.asarray(inputs['dst'])
    core = dst // NLOC
    dloc = dst % NLOC
    blk = dloc // P
    # table halves are chunked by LOCAL row (AllGather chunk = first/second
    # half of each core's rows): table row = owner*HALF_LOC + local_off
    HALF_LOC = NLOC // 2
    half = ((src % NLOC) >= HALF_LOC).astype(np.int64)
    # tile count per (core, blk, half); per-segment tile counts are the max
    # over cores so the single SPMD program fits every core's stream.
    cnt = np.zeros((NCORE, NB, 2), np.int64)
    np.add.at(cnt, (core, blk, half), 1)
    # seg order is half-major (half*NB + blk): each half's tiles stream
    # contiguously so gather batches can span dst-block boundaries
    segT = np.maximum(1, -(-cnt.max(axis=0) // P)).T.reshape(-1)  # [2*NB]
    offs = np.concatenate([[0], np.cumsum(segT)])               # tile offsets
    TOT = int(offs[-1])

    # shared (per-core identical) weights
    sh = {}
    for i in range(3):
        sh[f'Wemb{i}'] = np.ascontiguousarray(inputs[f'W_emb{i}']).astype(bf)
        sh[f'bemb{i}'] = np.asarray(inputs[f'b_emb{i}'],
                                    np.float32).reshape(-1, 1)
        wd = np.asarray(inputs[f'Wd{i}'], np.float32)
        sh[f'WdT4{i}'] = np.ascontiguousarray(
            np.concatenate([wd] * H, axis=0) * (1.0 / H)).astype(bf)
        sh[f'bd{i}'] = np.ascontiguousarray(
            np.asarray(inputs[f'bd{i}'], np.float32).reshape(-1, P).T)
    sh['fc0'] = np.asarray(inputs['fc0']).astype(bf)
    sh['res0'] = np.asarray(inputs['res0']).astype(bf)
    sh['fc1'] = np.asarray(inputs['fc1']).astype(bf)
    sh['res1'] = np.asarray(inputs['res1']).astype(bf)
    sh['fc1T'] = np.ascontiguousarray(np.asarray(inputs['fc1']).T).astype(bf)
    sh['fc0T'] = np.ascontiguousarray(np.asarray(inputs['fc0']).T).astype(bf)
    for li in (0, 1):
        al = np.asarray(inputs[f'al{li}'], np.float32)
        ar = np.asarray(inputs[f'ar{li}'], np.float32)
        Hh, D = al.shape
        blkm = np.zeros((Hh * D, 8), np.float32)
        for h in range(Hh):
            blkm[h * D:(h + 1) * D, h] = al[h]
            blkm[h * D:(h + 1) * D, 4 + h] = ar[h]
        sh[f'alr{li}'] = blkm.astype(bf)

    # per-core edge streams + one-hots
    per_core = []
    order_all = np.lexsort((dloc, blk, half, core))
    src_s = src[order_all]; dloc_s = dloc[order_all]
    blk_s = blk[order_all]; half_s = half[order_all]; core_s = core[order_all]
    core_off = np.searchsorted(core_s, np.arange(NCORE + 1))
    for c in range(NCORE):
        s0, s1 = core_off[c], core_off[c + 1]
        es, ed, eb, eh = (src_s[s0:s1], dloc_s[s0:s1], blk_s[s0:s1],
                          half_s[s0:s1])
        seg_id = eh * NB + eb
        # position within each (blk, half) group
        grp_start = np.searchsorted(seg_id, np.arange(NB * 2 + 1))
        pos = np.arange(len(es)) - grp_start[seg_id]
        spos = offs[seg_id] * P + pos
        total = TOT * P
        idx16 = np.zeros(total, np.int16)
        localidx = ((es // NLOC) * HALF_LOC
                    + (es % NLOC) % HALF_LOC).astype(np.int16)
        idx16[spos] = localidx
        # wrapped idx layout per (blk,half): [16, T*8] tiled to 128 rows
        idx_arr = np.zeros((P, TOT * 8), np.int16)
        for g in range(NB * 2):
            Tg = int(segT[g])
            w = idx16[offs[g] * P:offs[g + 1] * P].reshape(Tg * 8, 16).T
            idx_arr[:, offs[g] * 8:offs[g + 1] * 8] = np.tile(w, (8, 1))
        # one-hots
        oh = np.zeros((P, total), bf)
        ohT = np.zeros((P, total), bf)
        pp_ = spos
        t_of = pp_ // P
        e_of = pp_ % P
        dr = (ed - eb * P)
        oh[e_of, t_of * P + dr] = 1
        ohT[dr, t_of * P + e_of] = 1
        d = {'idx': idx_arr, 'oh': oh, 'ohT': ohT}
        r0, r1 = c * NLOC, (c + 1) * NLOC
        for i in range(3):
            d[f'featT{i}'] = np.ascontiguousarray(
                np.asarray(inputs[f'feat{i}'])[r0:r1].T).astype(bf)
        per_core.append(d)
    return sh, per_core, tuple(int(t) for t in segT)


_CACHE = {}


def _run(inputs, **kw):
    cfg = CFG
    sh, per_core, segT = _host_prep(inputs, cfg)
    key = ('v2', segT)
    if key not in _CACHE:
        _CACHE[key] = build_bass(cfg, segT)
    nc = _CACHE[key]
    in_maps = [{**sh, **pc} for pc in per_core]
    res = run_bass_kernel_spmd(nc, in_maps,
                               core_ids=list(range(cfg['NCORE'])), **kw)
    outs = []
    for i in range(3):
        outs.append(np.concatenate(
            [np.asarray(res.results[c][f'outT{i}'], np.float32).T
             for c in range(cfg['NCORE'])], axis=0))
    return tuple(outs), res


def kernel(**inputs):
    outs, _ = _run(inputs)
    return outs



# revision 48
# speedup vs baseline: 1.3764x; 1.0474x over previous
"""Trainium2 Bass kernel for CancerGATE (3-omics GAT autoencoder).

Sharding: nodes row-sharded across 8 NeuronCores. Dense phases (embedding,
projections, decode) run on each core's 6250-node shard in a transposed
layout (features on partitions, nodes on the free dim). The projected
features + attention-left logits are AllGathered into two per-core DRAM
table halves (chunked by local row so the second chunk's collective
overlaps edge processing of the first); the edge phase gathers source rows
by edge (dma_gather, int16 indices), weights them by the attention
coefficient (bf16 u written into the row's spare columns so one one-hot
matmul accumulates both the weighted-z sum and the softmax denominator)
and scatter-adds into per-destination-block PSUM via host-built one-hot
matmuls. The edge loop runs half-outer with a bf16 SBUF stash of half-0
partials. Per-(dst-block, half) edge-tile counts are the max over cores,
baked into the program (~6% less padding than a global max). Edge softmax
uses the unnormalized form (exp without max subtraction -- the logit range
for this model is [-3, 4]) so normalization is a single reciprocal per
destination node after aggregation.
"""
import sys
sys.path.insert(0, '/opt/trn_rl_repo')

import numpy as np
import ml_dtypes

import concourse.bass as bass
import concourse.bacc as bacc
import concourse.tile as tile
from concourse import mybir
from concourse.bass import IndirectOffsetOnAxis, broadcast_tensor_aps
from concourse.bass_utils import run_bass_kernel_spmd
from concourse.masks import make_identity

USE_INDIRECT = False  # HW layout of indirect gather differs from sim; using dma_gather
USE_PREP = False  # prepare_only+trigger races on HW even with cleared sems

F32 = mybir.dt.float32
BF16 = mybir.dt.bfloat16
F8 = mybir.dt.float8e4
I16 = mybir.dt.int16
AF = mybir.ActivationFunctionType
OP = mybir.AluOpType

P = 128
GB = 8  # tiles per dma_gather batch (1024 indices; >=1280 wedges the device)


def _dcfg(N=50000, NCORE=8, IN_DIM=512, D0=128, H=4, O0=128, O1=64, FW=512):
    c = {}
    c['N'] = N; c['NCORE'] = NCORE; c['IN_DIM'] = IN_DIM; c['D0'] = D0
    c['H'] = H; c['O0'] = O0; c['O1'] = O1
    c['CONCAT'] = 3 * D0
    c['Z0'] = H * O0
    c['Z1'] = H * O1
    c['DEC'] = c['CONCAT'] // H
    c['NLOC'] = N // NCORE
    c['NB'] = -(-c['NLOC'] // P)
    c['SPLIT'] = N // 2
    # table row widths in fp8 bytes: z (fp8) + 8 bf16 el vals (16 B),
    # padded to a 256 B multiple (dma_gather elem constraint)
    c['EXT0'] = -(-(c['Z0'] + 16) // 256) * 256
    c['EXT1'] = -(-(c['Z1'] + 16) // 256) * 256
    c['FW'] = FW
    return c


CFG = _dcfg()


def _chunks(total, w):
    return [(a, min(a + w, total)) for a in range(0, total, w)]


def _f2(ap):
    """Flatten a sliced 3D AP to 2D [P, cols]."""
    return ap.rearrange("p a b -> p (a b)")


def build_bass(cfg, segT):
    N, NCORE, NLOC, NB = cfg['N'], cfg['NCORE'], cfg['NLOC'], cfg['NB']
    IN_DIM, CONCAT, Z0, Z1, DEC = (cfg['IN_DIM'], cfg['CONCAT'], cfg['Z0'],
                                   cfg['Z1'], cfg['DEC'])
    EXT0, EXT1, H, FW = cfg['EXT0'], cfg['EXT1'], cfg['H'], cfg['FW']
    SPLIT = cfg['SPLIT']
    KI = IN_DIM // P      # k-chunks for IN_DIM contraction
    KC = CONCAT // P      # k-chunks for CONCAT
    K0 = Z0 // P          # z0 partition blocks
    K1 = Z1 // P          # z1 partition blocks
    KD = -(-DEC * H // P) # = KC
    LW = NLOC - (NB - 1) * P  # last node-block width
    HALF_LOC = NLOC // 2      # AllGather chunk boundary (local rows)
    nch = _chunks(NLOC, cfg['FW'])
    # per-(blk,half) segment tile counts (max over cores, host-computed)
    segT = list(segT)
    offs = [0]
    for t in segT:
        offs.append(offs[-1] + t)
    TOT = offs[-1]        # total edge tiles per core

    nc = bacc.Bacc("TRN2", target_bir_lowering=False, debug=False,
                   num_devices=NCORE)

    # ---------------- I/O ----------------
    ein = lambda nm, sh, dt: nc.dram_tensor(nm, sh, dt, kind="ExternalInput")
    featT = [ein(f"featT{i}", [IN_DIM, NLOC], BF16) for i in range(3)]
    Wemb = [ein(f"Wemb{i}", [IN_DIM, cfg['D0']], BF16) for i in range(3)]
    bemb = [ein(f"bemb{i}", [cfg['D0'], 1], F32) for i in range(3)]
    fc0 = ein("fc0", [CONCAT, Z0], BF16)
    res0 = ein("res0", [CONCAT, Z0], BF16)
    alr0 = ein("alr0", [Z0, 8], BF16)
    fc1 = ein("fc1", [Z0, Z1], BF16)
    res1 = ein("res1", [Z0, Z1], BF16)
    alr1 = ein("alr1", [Z1, 8], BF16)
    fc1T = ein("fc1T", [Z1, Z0], BF16)
    fc0T = ein("fc0T", [Z0, CONCAT], BF16)
    WdT4 = [ein(f"WdT4{i}", [CONCAT, IN_DIM], BF16) for i in range(3)]
    bd = [ein(f"bd{i}", [P, IN_DIM // P], F32) for i in range(3)]
    idx_d = ein("idx", [P, TOT * 8], I16)
    oh_d = ein("oh", [P, TOT * P], BF16)
    ohT_d = ein("ohT", [P, TOT * P], BF16)
    outT = [nc.dram_tensor(f"outT{i}", [IN_DIM, NLOC], F32,
                           kind="ExternalOutput") for i in range(3)]

    with tile.TileContext(nc) as tc:
        with (
            tc.tile_pool(name="wpool", bufs=1) as wp,
            tc.tile_pool(name="dram", bufs=1, space="DRAM") as dp,
            tc.tile_pool(name="persist", bufs=1) as pp,
        ):
            # gather DMA-completion sems for prepare_only mode. Tile only
            # clears ITS OWN sems at kernel tail, so across NEFF executions
            # (warmup + profiled run) a user sem keeps stale counts and
            # consumer waits release early -> garbage reads. Clear at start.
            gsem0 = nc.alloc_semaphore("gsem0")
            gsem1 = nc.alloc_semaphore("gsem1")
            grng = range(min(gsem0.num, gsem1.num),
                         max(gsem0.num, gsem1.num) + 1)
            nc.gpsimd.dma_reset(grng)
            nc.gpsimd.sem_clear(grng)

            # ------------- internal DRAM -------------
            aspace = "Shared" if NCORE > 4 else "Local"
            ag_in0 = dp.tile([NLOC, EXT0], F8)
            table0 = [dp.tile([SPLIT, EXT0], F8, addr_space=aspace,
                              name=f"table0_{i}") for i in range(2)]
            ag_in1 = dp.tile([NLOC, EXT1], F8)
            table1 = [dp.tile([SPLIT, EXT1], F8, addr_space=aspace,
                              name=f"table1_{i}") for i in range(2)]
            r0T_d = dp.tile([Z0, NLOC], BF16)
            r1T_d = dp.tile([Z1, NLOC], BF16)

            # ------------- persistent SBUF -------------
            ident = pp.tile([P, P], BF16)
            make_identity(nc, ident[:])
            er0_all = pp.tile([P, NB * 8], BF16)
            er1_all = pp.tile([P, NB * 8], BF16)
            nc.vector.memset(er0_all[:], 0.0)
            nc.vector.memset(er1_all[:], 0.0)

            # weights to SBUF (decode weights loaded late, in the decode
            # pool, so edge-phase pools can use the space)
            def wload(name, t, kparts, pool=None):
                w = (pool or wp).tile([P, kparts, t.shape[1]], BF16, name=name)
                nc.sync.dma_start(
                    w[:], t[:, :].rearrange("(k p) m -> p k m", p=P))
                return w
            Wemb_s = [wload(f"wemb{i}", Wemb[i], KI) for i in range(3)]
            fc0_s = wload("fc0s", fc0, KC)
            res0_s = wload("res0s", res0, KC)
            alr0_s = wload("alr0s", alr0, K0)
            fc1_s = wload("fc1s", fc1, K0)
            res1_s = wload("res1s", res1, K0)
            alr1_s = wload("alr1s", alr1, K1)
            bemb_s = []
            for i in range(3):
                b = wp.tile([P, 1], F32, name=f"bemb{i}s")
                nc.sync.dma_start(b[:], bemb[i][:, :])
                bemb_s.append(b)
            bd_s = []
            for i in range(3):
                b = wp.tile([P, IN_DIM // P], F32, name=f"bd{i}s")
                nc.sync.dma_start(b[:], bd[i][:, :])
                bd_s.append(b)

            # =========================================================
            # helper: dense matmul  outT_sb[:, mb, c0:c1] over chunks
            # =========================================================
            def dense(out_sb, sbuf_pool, lhs_sb, kparts, mblocks, rhs_sb,
                      psum_pool, act_fn, bias=None, out_f32_to=None,
                      name=""):
                """out[mb][P, chunk] = act( sum_k lhs[k].T @ rhs[k] + bias )"""
                for mb in range(mblocks):
                    for (a, b) in nch:
                        w = b - a
                        ps = psum_pool.tile([P, FW], F32, tag="dps")
                        for k in range(kparts):
                            nc.tensor.matmul(
                                ps[:, :w],
                                lhsT=lhs_sb[:, k, mb * P:(mb + 1) * P],
                                rhs=rhs_sb[:, k, a:b],
                                start=(k == 0), stop=(k == kparts - 1))
                        kw = {}
                        if bias is not None:
                            kw['bias'] = bias(mb)
                        if out_f32_to is not None:
                            o = sbuf_pool.tile([P, FW], F32, tag="dout")
                            nc.scalar.activation(o[:, :w], ps[:, :w],
                                                 act_fn, **kw)
                            nc.sync.dma_start(out_f32_to[mb * P:(mb + 1) * P,
                                                         a:b], o[:, :w])
                        else:
                            nc.scalar.activation(out_sb[:, mb, a:b],
                                                 ps[:, :w], act_fn, **kw)

            # =========================================================
            # D0: embedding -> hT  (CONCAT x NLOC, bf16, SBUF)
            # =========================================================
            with tc.tile_pool(name="d0psum", bufs=2, space="PSUM") as psp, \
                 tc.tile_pool(name="d0sb", bufs=3) as sbp:
                hT = pp.tile([P, KC, NLOC], BF16, name="hT", tag="hbuf")
                for fb in range(3):
                    for (a, b) in nch:
                        w = b - a
                        ps = psp.tile([P, FW], F32, tag="emb")
                        for k in range(KI):
                            rt = sbp.tile([P, FW], BF16, tag="feat")
                            nc.sync.dma_start(
                                rt[:, :w], featT[fb][k * P:(k + 1) * P, a:b])
                            nc.tensor.matmul(ps[:, :w],
                                             lhsT=Wemb_s[fb][:, k, :],
                                             rhs=rt[:, :w],
                                             start=(k == 0),
                                             stop=(k == KI - 1))
                        nc.scalar.activation(hT[:, fb, a:b], ps[:, :w],
                                             AF.Relu, bias=bemb_s[fb][:, :1])

            # =========================================================
            # D0b: z0T, el/er, r0T, table0 assembly, AllGather
            # =========================================================
            def proj_layer(h_sb, kparts, fc_sb, res_sb, alr_sb, zparts,
                           er_all, ag_in, table, rT_d, EXT, Z):
                with tc.tile_pool(name="p1psum", bufs=2, space="PSUM") as psp, \
                     tc.tile_pool(name="p1sb", bufs=3) as sbp, \
                     tc.tile_pool(name="p1z", bufs=1) as zp:
                    zT = zp.tile([P, zparts, NLOC], BF16, name="zT")
                    # zT = fc.T @ h
                    for zb in range(zparts):
                        for (a, b) in nch:
                            w = b - a
                            ps = psp.tile([P, FW], F32, tag="z")
                            for k in range(kparts):
                                nc.tensor.matmul(
                                    ps[:, :w],
                                    lhsT=fc_sb[:, k, zb * P:(zb + 1) * P],
                                    rhs=h_sb[:, k, a:b],
                                    start=(k == 0), stop=(k == kparts - 1))
                            nc.scalar.activation(zT[:, zb, a:b], ps[:, :w],
                                                 AF.Copy)
                    # el/er (8 rows) from zT
                    lr_hi = pp.tile([8, NLOC], BF16, name="lrhi", tag="lrhi")
                    lr_lo = pp.tile([8, NLOC], BF16, name="lrlo", tag="lrlo")
                    for (a, b) in nch:
                        w = b - a
                        ps = psp.tile([8, FW], F32, tag="lr")
                        for zb in range(zparts):
                            nc.tensor.matmul(ps[:, :w],
                                             lhsT=alr_sb[:, zb, :],
                                             rhs=zT[:, zb, a:b],
                                             start=(zb == 0),
                                             stop=(zb == zparts - 1))
                        nc.vector.tensor_copy(lr_hi[:, a:b], ps[:, :w])
                        nc.vector.tensor_tensor(lr_lo[:, a:b], ps[:, :w],
                                                lr_hi[:, a:b], op=OP.subtract)
                    # assemble node-major table rows + er_all
                    for nb in range(NB):
                        a = nb * P
                        w = min(P, NLOC - a)
                        stage = sbp.tile([P, EXT], F8, tag="stage")
                        sel = stage[:, Z:Z + 16].bitcast(BF16)  # 8 bf16 el
                        nc.vector.memset(stage[:, Z + 16:EXT], 0.0)
                        for zb in range(zparts):
                            pt = psp.tile([P, P], BF16, tag="tr")
                            nc.tensor.transpose(pt[:w, :], zT[:, zb, a:a + w],
                                                ident[:])
                            nc.vector.tensor_copy(
                                stage[:w, zb * P:(zb + 1) * P], pt[:w, :])
                        pt = psp.tile([P, 8], BF16, tag="tr")
                        nc.tensor.transpose(pt[:w, :], lr_hi[:, a:a + w],
                                            ident[:8, :8])
                        nc.vector.tensor_copy(sel[:w, 0:4], pt[:w, 0:4])
                        nc.vector.tensor_copy(er_all[:w, nb * 8:nb * 8 + 4],
                                              pt[:w, 4:8])
                        pt2 = psp.tile([P, 8], BF16, tag="tr")
                        nc.tensor.transpose(pt2[:w, :], lr_lo[:, a:a + w],
                                            ident[:8, :8])
                        nc.vector.tensor_copy(sel[:w, 4:8], pt2[:w, 0:4])
                        nc.vector.tensor_copy(
                            er_all[:w, nb * 8 + 4:nb * 8 + 8], pt2[:w, 4:8])
                        nc.sync.dma_start(ag_in[a:a + w, :], stage[:w, :])
                        if a < HALF_LOC <= a + w:
                            # first-half table chunk: lets the edge phase
                            # start on half-0 while the rest gathers
                            nc.gpsimd.collective_compute(
                                "AllGather", OP.bypass,
                                replica_groups=[list(range(NCORE))],
                                ins=[ag_in[:HALF_LOC, :]],
                                outs=[table[0][:]])
                    nc.gpsimd.collective_compute(
                        "AllGather", OP.bypass,
                        replica_groups=[list(range(NCORE))],
                        ins=[ag_in[HALF_LOC:, :]], outs=[table[1][:]])
                    # residual projection rT
                    for rb in range(zparts):
                        for (a, b) in nch:
                            w = b - a
                            ps = psp.tile([P, FW], F32, tag="r")
                            for k in range(kparts):
                                nc.tensor.matmul(
                                    ps[:, :w],
                                    lhsT=res_sb[:, k, rb * P:(rb + 1) * P],
                                    rhs=h_sb[:, k, a:b],
                                    start=(k == 0), stop=(k == kparts - 1))
                            ot = sbp.tile([P, FW], BF16, tag="rout")
                            nc.scalar.activation(ot[:, :w], ps[:, :w], AF.Copy)
                            nc.sync.dma_start(rT_d[rb * P:(rb + 1) * P, a:b],
                                              ot[:, :w])

            proj_layer(hT, KC, fc0_s, res0_s, alr0_s, K0, er0_all,
                       ag_in0, table0, r0T_d, EXT0, Z0)

            # =========================================================
            # E: edge phase (shared for both layers)
            # =========================================================
            def edge_phase(table, er_all, rT_d, h_out, relu, EXT, Z, gsem):
                zparts = Z // P
                with tc.tile_pool(name="epsum", bufs=2, space="PSUM") as psp, \
                     tc.tile_pool(name="esb", bufs=2) as sbp, \
                     tc.tile_pool(name="ezg", bufs=4) as zgp, \
                     tc.tile_pool(name="esb2", bufs=2) as sbp2:
                    # half-0 partial sums stash (bf16) so half-1 can run
                    # as a separate pass, overlapping the second AllGather
                    stash = pp.tile([P, NB, Z + 4], BF16, tag="stash")

                    def finish_blk(blk, half, psz):
                        """Stash (half 0) or combine+normalize+fused
                        untranspose (half 1) once a dst block's last edge
                        tile has been accumulated."""
                        if half == 0:
                            nc.vector.tensor_copy(stash[:, blk, :], psz[:])
                            return
                        tot = sbp.tile([P, Z + 4], F32, tag="tot")
                        nc.vector.tensor_tensor(tot[:], psz[:],
                                                stash[:, blk, :], op=OP.add)
                        sp = sbp.tile([P, 4], F32, tag="sp")
                        nc.vector.tensor_scalar_add(sp[:], tot[:, Z:Z + 4],
                                                    1e-9)
                        rp = sbp.tile([P, 4], F32, tag="rp")
                        nc.vector.reciprocal(rp[:], sp[:])
                        rt = sbp.tile([P, Z], BF16, tag="rstt")
                        hw = Z // H
                        for h in range(H):
                            nc.vector.tensor_scalar_mul(
                                rt[:, h * hw:(h + 1) * hw],
                                tot[:, h * hw:(h + 1) * hw],
                                rp[:, h:h + 1])
                        # fused untranspose: write this block straight into
                        # the next phase's [Z, NLOC] activation so downstream
                        # column-chunks can start while later blocks are
                        # still aggregating
                        a = blk * P
                        w = min(P, NLOC - a)
                        for zb in range(zparts):
                            pt = psp.tile([P, P], BF16, tag="trp")
                            nc.tensor.transpose(
                                pt[:, :w], rt[:w, zb * P:(zb + 1) * P],
                                ident[:w, :w])
                            rr = sbp.tile([P, P], BF16, tag="rr")
                            nc.sync.dma_start(
                                rr[:, :w], rT_d[zb * P:(zb + 1) * P, a:a + w])
                            nc.vector.tensor_tensor(
                                h_out[:, zb, a:a + w], pt[:, :w],
                                rr[:, :w], op=OP.add)
                            if relu:
                                nc.scalar.activation(
                                    h_out[:, zb, a:a + w],
                                    h_out[:, zb, a:a + w], AF.Relu)

                    for half in range(2):
                        # gather batches stream the whole half contiguously,
                        # ignoring dst-block boundaries (fewer, fuller
                        # dma_gather calls); psz accumulators start/stop at
                        # block transitions via the per-tile block map.
                        base = half * NB
                        blk_first = {}
                        blk_last = {}
                        tile_blk = []
                        g = 0
                        for blk in range(NB):
                            Tb = segT[base + blk]
                            blk_first[blk] = g
                            g += Tb
                            blk_last[blk] = g - 1
                            tile_blk += [blk] * Tb
                        THH = g
                        o = offs[base]
                        tbl = table[half][:]
                        psz_of = {}
                        for (t0, t1) in _chunks(THH, GB):
                            nt = t1 - t0
                            sz = nt * P
                            ohs = sbp2.tile([P, GB * P], BF16, tag="ohs")
                            nc.sync.dma_start(
                                ohs[:, :nt * P],
                                oh_d[:, (o + t0) * P:(o + t1) * P])
                            ohTs = sbp2.tile([P, GB * P], BF16, tag="ohTs")
                            nc.sync.dma_start(
                                ohTs[:, :nt * P],
                                ohT_d[:, (o + t0) * P:(o + t1) * P])
                            it = sbp.tile([P, GB * 8], I16, tag="idx")
                            nc.sync.dma_start(
                                it[:, :nt * 8],
                                idx_d[:, (o + t0) * 8:(o + t1) * 8])
                            zg = zgp.tile([P, GB, EXT], F8, tag="zg")
                            nc.gpsimd.dma_gather(
                                zg[:, :nt, :], tbl, it[:, :nt * 8],
                                sz, sz, EXT)
                            per = psp.tile([P, GB, 8], F32, tag="per")
                            for ts in range(nt):
                                b = tile_blk[t0 + ts]
                                nc.tensor.matmul(
                                    per[:, ts, :],
                                    lhsT=ohTs[:, ts * P:(ts + 1) * P],
                                    rhs=er_all[:, b * 8:b * 8 + 8],
                                    start=True, stop=True)
                            # u computation (batched over nt tiles); el
                            # rides the fp8 row as 8 bf16 vals at byte cols
                            # [Z:Z+16)
                            zgel = zg[:, :nt, Z:Z + 16].bitcast(BF16)
                            el = sbp.tile([P, GB, 4], F32, tag="el")
                            nc.vector.tensor_tensor(
                                el[:, :nt, :], zgel[:, :, 0:4],
                                zgel[:, :, 4:8], op=OP.add)
                            nc.vector.tensor_tensor(
                                el[:, :nt, :], el[:, :nt, :],
                                per[:, :nt, 0:4], op=OP.add)
                            nc.vector.tensor_tensor(
                                el[:, :nt, :], el[:, :nt, :],
                                per[:, :nt, 4:8], op=OP.add)
                            nc.vector.scalar_tensor_tensor(
                                el[:, :nt, :], el[:, :nt, :], 0.2,
                                el[:, :nt, :], op0=OP.mult, op1=OP.max)
                            # u: bf16 for the one-hot weighting (numerator),
                            # fp8 into zg cols [Z:Z+4) so the pss matmul
                            # accumulates sum(u) per dst.
                            uf = sbp.tile([P, GB, 4], BF16, tag="uf")
                            nc.scalar.activation(
                                uf[:, :nt, :].rearrange("p a b -> p (a b)"),
                                el[:, :nt, :].rearrange("p a b -> p (a b)"),
                                AF.Exp)
                            nc.scalar.activation(
                                zg[:, :nt, Z:Z + 4], el[:, :nt, :], AF.Exp)
                            # weight the ONE-HOT by u per head (bf16, keeps
                            # DVE off the fp8 path); the psz matmuls then run
                            # mixed bf16 lhsT x fp8 rhs per head.
                            hw = Z // H
                            oh3 = ohs[:, :nt * P].rearrange(
                                "p (t c) -> p t c", c=P)
                            ohw = sbp.tile([P, GB, H, P], BF16, tag="ohw")
                            for h in range(H):
                                i0, i1 = broadcast_tensor_aps(
                                    oh3, uf[:, :nt, h:h + 1])
                                nc.vector.tensor_tensor(
                                    ohw[:, :nt, h, :], i0, i1, op=OP.mult)
                            for ts in range(nt):
                                gi = t0 + ts
                                b = tile_blk[gi]
                                if gi == blk_first[b]:
                                    psz_of[b] = psp.tile(
                                        [P, Z + 4], F32, tag="psz",
                                        name=f"psz_h{half}_b{b}")
                                psz = psz_of[b]
                                first = (gi == blk_first[b])
                                last = (gi == blk_last[b])
                                for h in range(H):
                                    nc.tensor.matmul(
                                        psz[:, h * hw:(h + 1) * hw],
                                        lhsT=ohw[:, ts, h, :],
                                        rhs=zg[:, ts, h * hw:(h + 1) * hw],
                                        start=first, stop=last)
                                nc.tensor.matmul(
                                    psz[:, Z:Z + 4],
                                    lhsT=ohs[:, ts * P:(ts + 1) * P],
                                    rhs=zg[:, ts, Z:Z + 4],
                                    start=first, stop=last)
                                if last:
                                    finish_blk(b, half, psz)
                                    del psz_of[b]

            h1T = pp.tile([P, K0, NLOC], BF16, name="h1T", tag="hbuf")
            edge_phase(table0, er0_all, r0T_d, h1T, True, EXT0, Z0, gsem0)

            # ============== D1 + E1 ==============
            proj_layer(h1T, K0, fc1_s, res1_s, alr1_s, K1, er1_all,
                       ag_in1, table1, r1T_d, EXT1, Z1)
            d0T = pp.tile([P, K1, NLOC], BF16, name="d0T", tag="hbuf")
            edge_phase(table1, er1_all, r1T_d, d0T, False, EXT1, Z1, gsem1)

            # ============== decode ==============
            with tc.tile_pool(name="decp", bufs=2, space="PSUM") as psp, \
                 tc.tile_pool(name="decs", bufs=3) as sbp, \
                 tc.tile_pool(name="dwp", bufs=1) as dwp:
                fc1T_s = wload("fc1Ts", fc1T, K1, pool=dwp)
                fc0T_s = wload("fc0Ts", fc0T, K0, pool=dwp)
                WdT4_s = [wload(f"wdt{i}", WdT4[i], KC, pool=dwp)
                          for i in range(3)]
                # relu on d0T in-place
                for zb in range(K1):
                    for (a, b) in nch:
                        nc.scalar.activation(d0T[:, zb, a:b], d0T[:, zb, a:b],
                                             AF.Relu)
                d1T = dwp.tile([P, K0, NLOC], BF16, name="d1T")
                dense(d1T, sbp, fc1T_s, K1, K0, d0T, psp, AF.Relu, name="d1")
                d2T = pp.tile([P, KC, NLOC], BF16, name="d2T", tag="hbuf")
                dense(d2T, sbp, fc0T_s, K0, KC, d1T, psp, AF.Relu, name="d2")
                for i in range(3):
                    dense(None, sbp, WdT4_s[i], KC, IN_DIM // P, d2T, psp,
                          AF.Sigmoid, bias=lambda mb, i=i: bd_s[i][:, mb:mb + 1],
                          out_f32_to=outT[i], name=f"o{i}")

    nc.compile()
    return nc


# =====================================================================
# Host side
# =====================================================================

def _host_prep(inputs, cfg):
    N, NCORE, NLOC, NB = cfg['N'], cfg['NCORE'], cfg['NLOC'], cfg['NB']
    SPLIT, H = cfg['SPLIT'], cfg['H']
    bf = ml_dtypes.bfloat16
    src = np.asarray(inputs['src']); dst = np.asarray(inputs['dst'])
    core = dst // NLOC
    dloc = dst % NLOC
    blk = dloc // P
    # table halves are chunked by LOCAL row (AllGather chunk = first/second
    # half of each core's rows): table row = owner*HALF_LOC + local_off
    HALF_LOC = NLOC // 2
    half = ((src % NLOC) >= HALF_LOC).astype(np.int64)
    # tile count per (core, blk, half); per-segment tile counts are the max
    # over cores so the single SPMD program fits every core's stream.
    cnt = np.zeros((NCORE, NB, 2), np.int64)
    np.add.at(cnt, (core, blk, half), 1)
    # seg order is half-major (half*NB + blk): each half's tiles stream
    # contiguously so gather batches can span dst-block boundaries
    segT = np.maximum(1, -(-cnt.max(axis=0) // P)).T.reshape(-1)  # [2*NB]
    offs = np.concatenate([[0], np.cumsum(segT)])               # tile offsets
    TOT = int(offs[-1])

    # shared (per-core identical) weights
    sh = {}
    for i in range(3):
        sh[f'Wemb{i}'] = np.ascontiguousarray(inputs[f'W_emb{i}']).astype(bf)
        sh[f'bemb{i}'] = np.asarray(inputs[f'b_emb{i}'],
                                    np.float32).reshape(-1, 1)
        wd = np.asarray(inputs[f'Wd{i}'], np.float32)
        sh[f'WdT4{i}'] = np.ascontiguousarray(
            np.concatenate([wd] * H, axis=0) * (1.0 / H)).astype(bf)
        sh[f'bd{i}'] = np.ascontiguousarray(
            np.asarray(inputs[f'bd{i}'], np.float32).reshape(-1, P).T)
    sh['fc0'] = np.asarray(inputs['fc0']).astype(bf)
    sh['res0'] = np.asarray(inputs['res0']).astype(bf)
    sh['fc1'] = np.asarray(inputs['fc1']).astype(bf)
    sh['res1'] = np.asarray(inputs['res1']).astype(bf)
    sh['fc1T'] = np.ascontiguousarray(np.asarray(inputs['fc1']).T).astype(bf)
    sh['fc0T'] = np.ascontiguousarray(np.asarray(inputs['fc0']).T).astype(bf)
    for li in (0, 1):
        al = np.asarray(inputs[f'al{li}'], np.float32)
        ar = np.asarray(inputs[f'ar{li}'], np.float32)
        Hh, D = al.shape
        blkm = np.zeros((Hh * D, 8), np.float32)
        for h in range(Hh):
            blkm[h * D:(h + 1) * D, h] = al[h]
            blkm[h * D:(h + 1) * D, 4 + h] = ar[h]
        sh[f'alr{li}'] = blkm.astype(bf)

    # per-core edge streams + one-hots
    per_core = []
    order_all = np.lexsort((dloc, blk, half, core))
    src_s = src[order_all]; dloc_s = dloc[order_all]
    blk_s = blk[order_all]; half_s = half[order_all]; core_s = core[order_all]
    core_off = np.searchsorted(core_s, np.arange(NCORE + 1))
    for c in range(NCORE):
        s0, s1 = core_off[c], core_off[c + 1]
        es, ed, eb, eh = (src_s[s0:s1], dloc_s[s0:s1], blk_s[s0:s1],
                          half_s[s0:s1])
        seg_id = eh * NB + eb
        # position within each (blk, half) group
        grp_start = np.searchsorted(seg_id, np.arange(NB * 2 + 1))
        pos = np.arange(len(es)) - grp_start[seg_id]
        spos = offs[seg_id] * P + pos
        total = TOT * P
        idx16 = np.zeros(total, np.int16)
        localidx = ((es // NLOC) * HALF_LOC
                    + (es % NLOC) % HALF_LOC).astype(np.int16)
        idx16[spos] = localidx
        # wrapped idx layout per (blk,half): [16, T*8] tiled to 128 rows
        idx_arr = np.zeros((P, TOT * 8), np.int16)
        for g in range(NB * 2):
            Tg = int(segT[g])
            w = idx16[offs[g] * P:offs[g + 1] * P].reshape(Tg * 8, 16).T
            idx_arr[:, offs[g] * 8:offs[g + 1] * 8] = np.tile(w, (8, 1))
        # one-hots
        oh = np.zeros((P, total), bf)
        ohT = np.zeros((P, total), bf)
        pp_ = spos
        t_of = pp_ // P
        e_of = pp_ % P
        dr = (ed - eb * P)
        oh[e_of, t_of * P + dr] = 1
        ohT[dr, t_of * P + e_of] = 1
        d = {'idx': idx_arr, 'oh': oh, 'ohT': ohT}
        r0, r1 = c * NLOC, (c + 1) * NLOC
        for i in range(3):
            d[f'featT{i}'] = np.ascontiguousarray(
                np.asarray(inputs[f'feat{i}'])[r0:r1].T).astype(bf)
        per_core.append(d)
    return sh, per_core, tuple(int(t) for t in segT)


_CACHE = {}


def _run(inputs, **kw):
    cfg = CFG
    sh, per_core, segT = _host_prep(inputs, cfg)
    key = ('v2', segT)
    if key not in _CACHE:
        _CACHE[key] = build_bass(cfg, segT)
    nc = _CACHE[key]
    in_maps = [{**sh, **pc} for pc in per_core]
    res = run_bass_kernel_spmd(nc, in_maps,
                               core_ids=list(range(cfg['NCORE'])), **kw)
    outs = []
    for i in range(3):
        outs.append(np.concatenate(
            [np.asarray(res.results[c][f'outT{i}'], np.float32).T
             for c in range(cfg['NCORE'])], axis=0))
    return tuple(outs), res


def kernel(**inputs):
    outs, _ = _run(inputs)
    return outs

